# revision 34
# baseline (speedup 1.0000x reference)
"""Trainium2 Bass kernel for the 2-layer hyperbolic (Poincare ball) GCN encoder.

Strategy (8 NeuronCores, SPMD), v2 pipelined:
  - Nodes sharded across cores (2500 rows/core, padded to 2560 = 20 tiles of 128).
  - Weights replicated (bf16); dense mobius ops on the owned shard with all
    per-row reductions as [128, T] scalar grids (sum-of-squares on DVE via
    tensor_tensor_reduce to avoid ACT table thrash).
  - Tangent features (pre-scaled by deg^-0.5 on the source side) are
    AllGathered in bf16 in 4 quarter-chunks per layer (quarter-major ts_full
    layout) so the collective overlaps phase-A tails / previous-layer work.
  - Edges partitioned by destination, block-aligned gathers (<=8 chunks of
    128 edges per dma_gather); per-gather trailing -1 indices skip padding
    DMA; num_idxs_reg carries the SPMD-common valid count.
  - Segment-sum on TensorE via 0/1 selection matrices (is_equal) accumulated
    in PSUM; per-5-block grouped expmap epilogue (batched ACT scalars).
  - Layer 1's whole phase A (matvec + mobius scalar stages + ts stores + AG
    chunks) is emitted inside layer 0's phase-B block loop for overlap.
"""
import numpy as np
import ml_dtypes

import concourse.bass as bass
import concourse.bacc as bacc
import concourse.tile as tile
import concourse.mybir as mybir
from concourse.bass_utils import run_bass_kernel_spmd
from concourse.masks import make_identity

NCORES = 8
P = 128
NQ = 4               # AllGather chunks per layer
CPG = 8              # max chunks (of 128 edges) per dma_gather
MN = 1.0 - 4e-3
EPS = 1e-15
ATEPS = 1e-7

f32 = mybir.dt.float32
bf16 = mybir.dt.bfloat16
i16 = mybir.dt.int16
AF = mybir.ActivationFunctionType
OP = mybir.AluOpType

_prog_cache = {}


# ----------------------------------------------------------------- host side

def _np_expmap0(u):
    u = np.asarray(u, np.float32)
    n = max(float(np.linalg.norm(u)), EPS)
    v = (np.tanh(n) * u / n).astype(np.float32)
    nn = max(float(np.linalg.norm(v)), EPS)
    if nn > MN:
        v = (v / nn * MN).astype(np.float32)
    return v


def _host_prep(x, edge_index):
    x = np.asarray(x, np.float32)
    ei = np.asarray(edge_index)
    N, D = x.shape
    assert N % NCORES == 0
    n_loc = N // NCORES
    T = (n_loc + P - 1) // P
    n_pad = T * P
    assert T % NQ == 0
    TQ = T // NQ              # tiles per AG quarter
    NPQ = TQ * P              # rows per AG quarter
    assert NCORES * n_pad <= 32767, "indices must fit int16"

    loops = np.arange(N, dtype=ei.dtype)
    ei = np.concatenate([ei, np.stack([loops, loops])], axis=1)
    row, col = ei[0].astype(np.int64), ei[1].astype(np.int64)
    deg = np.bincount(col, minlength=N).astype(np.float32)
    dis = (deg ** -0.5).astype(np.float32)

    # global source index within source-half tensors
    # ts_full_h = [NCORES, NPH, D]; NPH = n_pad // 2
    NPH = n_pad // 2
    r_s = row // n_loc
    i_s = row % n_loc
    h_s = i_s // NPH
    gsrc = r_s * NPH + (i_s % NPH)

    dst_core = col // n_loc
    dst_blk = (col % n_loc) // P
    dst_rel = (col % n_loc) % P

    # per (core, block, src-half) edge lists
    edges = [[[None, None] for _ in range(T)] for _ in range(NCORES)]
    for r in range(NCORES):
        sel = dst_core == r
        gb = dst_blk[sel]
        gh = h_s[sel]
        gs = gsrc[sel]
        gr = dst_rel[sel]
        order = np.lexsort((gh, gb))
        gb, gh, gs, gr = gb[order], gh[order], gs[order], gr[order]
        key = gb * 2 + gh
        bounds = np.searchsorted(key, np.arange(2 * T + 1))
        for b in range(T):
            for h in range(2):
                lo, hi = bounds[b * 2 + h], bounds[b * 2 + h + 1]
                so = np.argsort(gs[lo:hi], kind="stable")  # src-sorted: HBM
                edges[r][b][h] = (gs[lo:hi][so], gr[lo:hi][so])  # locality

    L = np.array([[[len(edges[r][b][h][0]) for h in range(2)]
                   for b in range(T)] for r in range(NCORES)], np.int64)
    # chunks per (block, half); block's chunk list = half0 chunks + half1
    CBH = [[int(np.ceil(L[:, b, h].max() / P)) for h in range(2)]
           for b in range(T)]
    CB = [CBH[b][0] + CBH[b][1] for b in range(T)]

    # gathers of <= CPG chunks per (block, half): (b, h, j, num_idxs, cg)
    gplan = []
    for b in range(T):
        for h in range(2):
            for j in range((CBH[b][h] + CPG - 1) // CPG):
                nj = min(CPG, CBH[b][h] - CPG * j)
                num_idxs = nj * P
                vr = np.clip(L[:, b, h] - CPG * P * j, 0, num_idxs)
                cg = int(vr.max())
                gplan.append((b, h, j, num_idxs, cg))

    tot16 = sum(g[3] // 16 for g in gplan)
    sumCB = sum(CB)

    idx_w = np.zeros((NCORES, 128, tot16), np.int16)
    edst = np.full((NCORES, P, sumCB), -1.0, np.float32)
    eoff = np.concatenate([[0], np.cumsum(CB)]).astype(int)

    for r in range(NCORES):
        col16 = 0
        for (b, h, j, num_idxs, cg) in gplan:
            gs, gr = edges[r][b][h]
            lo = CPG * P * j
            vr = int(np.clip(len(gs) - lo, 0, num_idxs))
            lin = np.full(num_idxs, -1, np.int64)
            lin[:vr] = gs[lo:lo + vr]
            lin[vr:cg] = 0                      # dummy valid rows
            w = lin.reshape(num_idxs // 16, 16).T.astype(np.int16)
            nc16 = num_idxs // 16
            idx_w[r][:, col16:col16 + nc16] = np.tile(w, (8, 1))
            col16 += nc16
        for b in range(T):
            nch = CB[b]
            dpad = np.full(nch * P, -1.0, np.float32)
            gs0, gr0 = edges[r][b][0]
            gs1, gr1 = edges[r][b][1]
            dpad[:len(gr0)] = gr0.astype(np.float32)
            off1 = CBH[b][0] * P
            dpad[off1:off1 + len(gr1)] = gr1.astype(np.float32)
            edst[r][:, eoff[b]:eoff[b] + nch] = dpad.reshape(nch, P).T

    dis_loc = np.zeros((NCORES, P, T), np.float32)
    for r in range(NCORES):
        d = np.zeros(n_pad, np.float32)
        d[:n_loc] = dis[r * n_loc:(r + 1) * n_loc]
        dis_loc[r] = d.reshape(T, P).T

    x_loc = np.zeros((NCORES, n_pad, D), np.float32)
    for r in range(NCORES):
        x_loc[r, :n_loc] = x[r * n_loc:(r + 1) * n_loc]

    iota = np.tile(np.arange(P, dtype=np.float32)[None, :], (P, 1))
    meta = dict(N=N, D=D, n_loc=n_loc, T=T, CB=tuple(CB),
                CBH=tuple(tuple(c) for c in CBH),
                gplan=tuple(gplan), n_pad=n_pad, NPQ=NPQ)
    per_core = [dict(x=x_loc[r], dis=dis_loc[r], gidx=idx_w[r],
                     edst=edst[r].astype(ml_dtypes.bfloat16),
                     iota=iota.astype(ml_dtypes.bfloat16))
                for r in range(NCORES)]
    return meta, per_core


# --------------------------------------------------------------- device side

def _build_program(T, CB, CBH, gplan, DC):
    D = DC * P
    NPAD = T * P
    NPH = NPAD // 2
    TQ = T // NQ
    NPQ = TQ * P
    EX = bf16
    G = 5                      # phase-B epilogue group (blocks)
    assert T % G == 0 and TQ == G
    sumCB = sum(CB)
    eoff = [0]
    for c in CB:
        eoff.append(eoff[-1] + c)
    tot16 = sum(g[3] // 16 for g in gplan)
    CPGH = max(max(ch) for ch in CBH)      # chunks per (block, half)

    nc = bacc.Bacc("TRN2", target_bir_lowering=False, debug=False,
                   num_devices=NCORES, num_swdge_queues=4,
                   dynamic_dma_scratch_size=16384)

    x_in = nc.dram_tensor("x", [NPAD, D], f32, kind="ExternalInput")
    wt_in = nc.dram_tensor("wt", [2, D, D], bf16, kind="ExternalInput")
    y_in = nc.dram_tensor("y", [2, P, D], f32, kind="ExternalInput")
    iota_in = nc.dram_tensor("iota", [P, P], bf16, kind="ExternalInput")
    dis_in = nc.dram_tensor("dis", [P, T], f32, kind="ExternalInput")
    gidx_in = nc.dram_tensor("gidx", [P, tot16], i16, kind="ExternalInput")
    edst_in = nc.dram_tensor("edst", [P, sumCB], bf16, kind="ExternalInput")
    out_ext = nc.dram_tensor("out", [NPAD, D], f32, kind="ExternalOutput")

    with tile.TileContext(nc) as tc:
        with (
            tc.tile_pool(name="const", bufs=1) as constp,
            tc.tile_pool(name="grid", bufs=1) as gridp,
            tc.tile_pool(name="big", bufs=1) as bigp,
            tc.tile_pool(name="work", bufs=3) as workp,
            tc.tile_pool(name="junk", bufs=3) as junkp,
            tc.tile_pool(name="msgs", bufs=5) as msgp,
            tc.tile_pool(name="sblk", bufs=4) as sblkp,
            tc.tile_pool(name="psT", bufs=1, space="PSUM") as psTp,
            tc.tile_pool(name="psM", bufs=2, space="PSUM") as psMp,
            tc.tile_pool(name="psA", bufs=5, space="PSUM") as psAp,
            tc.tile_pool(name="dram", bufs=1, space="DRAM") as dramp,
        ):
            # ---- constants ----
            wt_sb = constp.tile([P, 2 * DC * D], bf16, name="wt", tag="wt")
            for l in range(2):
                for k in range(DC):
                    nc.sync.dma_start(
                        out=wt_sb[:, (l * DC + k) * D:(l * DC + k + 1) * D],
                        in_=wt_in[l, k * P:(k + 1) * P, :])
            y_sb = constp.tile([P, 2 * D], f32, name="y", tag="y")
            nc.sync.dma_start(out=y_sb[:, 0:D], in_=y_in[0])
            nc.sync.dma_start(out=y_sb[:, D:2 * D], in_=y_in[1])
            iota_sb = constp.tile([P, P], bf16, name="iota", tag="iota")
            nc.sync.dma_start(out=iota_sb[:], in_=iota_in[:, :])
            ident = constp.tile([P, P], f32, name="ident", tag="ident")
            make_identity(nc, ident[:])
            disg = constp.tile([P, T], f32, name="dis", tag="dis")
            nc.sync.dma_start(out=disg[:], in_=dis_in[:, :])
            gidx_sb = constp.tile([P, tot16], i16, name="gidx", tag="gidx")
            nc.sync.dma_start(out=gidx_sb[:], in_=gidx_in[:, :])
            edst_sb = constp.tile([P, sumCB], bf16, name="edst", tag="edst")
            nc.sync.dma_start(out=edst_sb[:], in_=edst_in[:, :])

            # message tiles: memset all bufs once (trailing-skip leaves stale
            # bytes; they are S-masked but must never be NaN)
            mprev = []
            for i in range(5):
                m0 = msgp.tile([P, CPG * D], EX, name="m", tag="m")
                nc.gpsimd.memset(m0[:], 0.0)
                mprev.append(m0)

            # ---- persistent big tensors ----
            h_grid = bigp.tile([P, T * D], f32, name="h", tag="h")  # h then u
            agg_grid = bigp.tile([P, T * D], bf16, name="agg", tag="agg")
            hn2 = gridp.tile([P, T], f32, name="hn2", tag="hn2")

            def G_(tag):
                return gridp.tile([P, T], f32, name=tag, tag=tag)

            def tsl(t):
                return slice(t * D, (t + 1) * D)

            def sq_accum(src_ap, accum_ap, eng=None):
                """accum = sum(src*src) along free axis, on DVE (or eng)."""
                jj = junkp.tile([P, D], f32, name="junk", tag="junk")
                (eng or nc.vector).scalar_tensor_tensor(
                    out=jj[:], in0=src_ap, scalar=1.0, in1=src_ap,
                    op0=OP.mult, op1=OP.mult, accum_out=accum_ap)

            # per-layer DRAM tensors; ts_full split by source half so each
            # Shared tensor has exactly one AllGather writer
            ts_loc = [dramp.tile([NPAD, D], EX, name="tsl%d" % l,
                                 tag="tsl%d" % l) for l in range(2)]
            ts_full = [[dramp.tile([NCORES * NPH, D], EX, addr_space="Shared",
                                   name="tsf%d_%d" % (l, h),
                                   tag="tsf%d_%d" % (l, h))
                        for h in range(2)] for l in range(2)]

            mxn2_g = [G_("mxn2_0"), G_("mxn2_1")]
            y2col = gridp.tile([P, 2], f32, name="y2col", tag="y2col")

            def emit_pass1(l, t):
                pt = psTp.tile([P, D], f32, name="pt", tag="pt")
                for k in range(DC):
                    nc.tensor.transpose(
                        out=pt[:, k * P:(k + 1) * P],
                        in_=h_grid[:, t * D + k * P: t * D + (k + 1) * P],
                        identity=ident[:])
                hT = workp.tile([P, D], bf16, name="hT", tag="hT")
                nc.scalar.copy(hT[:], pt[:])
                pm = psMp.tile([P, D], f32, name="pm", tag="pm")
                for k in range(DC):
                    nc.tensor.matmul(
                        pm[:],
                        lhsT=hT[:, k * P:(k + 1) * P],
                        rhs=wt_sb[:, (l * DC + k) * D:(l * DC + k + 1) * D],
                        start=(k == 0), stop=(k == DC - 1))
                nc.scalar.copy(agg_grid[:, tsl(t)], pm[:])
                sq_accum(agg_grid[:, tsl(t)], mxn2_g[l][:, t:t + 1])

            def artanh2(nm, xx, cs):
                """grid of 2*artanh(clip(xx)) over column slice cs"""
                xcl = G_(nm + "_xcl")
                nc.vector.tensor_scalar_min(xcl[:, cs], xx[:, cs], 1.0 - ATEPS)
                a1 = G_(nm + "_a1")
                nc.scalar.activation(a1[:, cs], xcl[:, cs], AF.Ln,
                                     bias=1.0, scale=1.0)
                omx = G_(nm + "_omx")
                nc.vector.tensor_scalar(out=omx[:, cs], in0=xcl[:, cs],
                                        scalar1=-1.0, scalar2=1.0,
                                        op0=OP.mult, op1=OP.add)
                a2 = G_(nm + "_a2")
                nc.scalar.activation(a2[:, cs], omx[:, cs], AF.Ln)
                at2 = G_(nm + "_at2")
                nc.vector.tensor_tensor(out=at2[:, cs], in0=a1[:, cs],
                                        in1=a2[:, cs], op=OP.subtract)
                return at2

            pending_ag = []

            def flush_ag():
                while pending_ag:
                    pending_ag.pop(0)()

            def phase_A_stage(l, q, defer_ag=False):
                """mobius-add scalar stages + passes 2/3 + ts stores + AG
                chunk, for tiles [5q, 5q+5) of layer l."""
                cs = slice(q * G, (q + 1) * G)
                trng = range(q * G, (q + 1) * G)
                y_ap = y_sb[:, l * D:(l + 1) * D]
                mxn2 = mxn2_g[l]
                if q == 0:
                    sq_accum(y_ap, y2col[:, l:l + 1])
                # stage 1
                xn = G_("xn")
                nc.scalar.activation(xn[:, cs], hn2[:, cs], AF.Sqrt)
                mxn = G_("mxn")
                nc.scalar.activation(mxn[:, cs], mxn2[:, cs], AF.Sqrt)
                xng = G_("xng")
                nc.vector.tensor_scalar_max(xng[:, cs], xn[:, cs], EPS)
                xrec = G_("xrec")
                nc.vector.reciprocal(xrec[:, cs], xng[:, cs])
                at2 = artanh2("s1", xn, cs)
                rr2 = G_("rr2")
                nc.vector.tensor_tensor(out=rr2[:, cs], in0=at2[:, cs],
                                        in1=xrec[:, cs], op=OP.mult)
                mxng = G_("mxng")
                nc.vector.tensor_scalar_max(mxng[:, cs], mxn[:, cs], EPS)
                mrec = G_("mrec")
                nc.vector.reciprocal(mrec[:, cs], mxng[:, cs])
                cc = G_("cc")
                nc.vector.scalar_tensor_tensor(out=cc[:, cs], in0=mxn[:, cs],
                                               scalar=0.5, in1=rr2[:, cs],
                                               op0=OP.mult, op1=OP.mult)
                tch = G_("tch")
                nc.scalar.activation(tch[:, cs], cc[:, cs], AF.Tanh)
                tcg = G_("tcg")
                nc.vector.tensor_scalar_max(tcg[:, cs], tch[:, cs], EPS)
                tcrec = G_("tcrec")
                nc.vector.reciprocal(tcrec[:, cs], tcg[:, cs])
                psA_ = G_("psA")
                nc.vector.tensor_scalar(out=psA_[:, cs], in0=tcrec[:, cs],
                                        scalar1=MN, scalar2=1.0,
                                        op0=OP.mult, op1=OP.min)
                sp0 = G_("sp0")
                nc.vector.tensor_tensor(out=sp0[:, cs], in0=tch[:, cs],
                                        in1=mrec[:, cs], op=OP.mult)
                spg = G_("spg")
                nc.vector.tensor_tensor(out=spg[:, cs], in0=sp0[:, cs],
                                        in1=psA_[:, cs], op=OP.mult)
                tcm = G_("tcm")
                nc.vector.tensor_scalar_min(tcm[:, cs], tch[:, cs], MN)
                x2 = G_("x2")
                nc.vector.tensor_tensor(out=x2[:, cs], in0=tcm[:, cs],
                                        in1=tcm[:, cs], op=OP.mult)
                # pass 2: xy = sum((sp*mx) . y)
                xy = G_("xy")
                for t in trng:
                    jx = junkp.tile([P, D], f32, name="junk", tag="junk")
                    nc.vector.scalar_tensor_tensor(
                        out=jx[:], in0=agg_grid[:, tsl(t)],
                        scalar=spg[:, t:t + 1], in1=y_ap,
                        op0=OP.mult, op1=OP.mult, accum_out=xy[:, t:t + 1])
                # stage 2
                t0 = G_("t0")
                nc.vector.tensor_scalar(out=t0[:, cs], in0=xy[:, cs],
                                        scalar1=2.0, scalar2=1.0,
                                        op0=OP.mult, op1=OP.add)
                ag = G_("ag")
                nc.vector.tensor_scalar_add(ag[:, cs], t0[:, cs],
                                            y2col[:, l:l + 1])
                d0 = G_("d0")
                nc.vector.tensor_scalar_mul(d0[:, cs], x2[:, cs],
                                            y2col[:, l:l + 1])
                d1 = G_("d1")
                nc.vector.tensor_tensor(out=d1[:, cs], in0=d0[:, cs],
                                        in1=t0[:, cs], op=OP.add)
                dg = G_("dg")
                nc.vector.tensor_scalar_max(dg[:, cs], d1[:, cs], EPS)
                dinv = G_("dinv")
                nc.vector.reciprocal(dinv[:, cs], dg[:, cs])
                alpha = G_("alpha")
                nc.vector.tensor_tensor(out=alpha[:, cs], in0=ag[:, cs],
                                        in1=dinv[:, cs], op=OP.mult)
                bsc = G_("bsc")
                nc.vector.tensor_scalar(out=bsc[:, cs], in0=x2[:, cs],
                                        scalar1=-1.0, scalar2=1.0,
                                        op0=OP.mult, op1=OP.add)
                beta = G_("beta")
                nc.vector.tensor_tensor(out=beta[:, cs], in0=bsc[:, cs],
                                        in1=dinv[:, cs], op=OP.mult)
                alphasp = G_("alphasp")
                nc.vector.tensor_tensor(out=alphasp[:, cs], in0=alpha[:, cs],
                                        in1=spg[:, cs], op=OP.mult)
                # pass 3: u = alphasp*mx + beta*y (into h_grid)
                for t in trng:
                    t1 = workp.tile([P, D], f32, name="t1", tag="t1")
                    nc.vector.tensor_scalar_mul(t1[:], y_ap, beta[:, t:t + 1])
                    us = h_grid[:, tsl(t)]
                    nc.vector.scalar_tensor_tensor(
                        out=us, in0=agg_grid[:, tsl(t)],
                        scalar=alphasp[:, t:t + 1], in1=t1[:],
                        op0=OP.mult, op1=OP.add)
                # |u|^2 analytically: asp^2*|mx|^2 + 2*a*b*(sp*mx.y) + b^2*|y|^2
                un2 = G_("un2")
                ua = G_("ua")
                nc.vector.tensor_tensor(out=ua[:, cs], in0=alphasp[:, cs],
                                        in1=alphasp[:, cs], op=OP.mult)
                ub = G_("ub")
                nc.vector.tensor_tensor(out=ub[:, cs], in0=ua[:, cs],
                                        in1=mxn2[:, cs], op=OP.mult)
                uc = G_("uc")
                nc.vector.tensor_tensor(out=uc[:, cs], in0=alpha[:, cs],
                                        in1=beta[:, cs], op=OP.mult)
                ud = G_("ud")
                nc.vector.tensor_tensor(out=ud[:, cs], in0=uc[:, cs],
                                        in1=xy[:, cs], op=OP.mult)
                ue = G_("ue")
                nc.vector.tensor_tensor(out=ue[:, cs], in0=beta[:, cs],
                                        in1=beta[:, cs], op=OP.mult)
                uf = G_("uf")
                nc.vector.tensor_scalar_mul(uf[:, cs], ue[:, cs],
                                            y2col[:, l:l + 1])
                ug = G_("ug")
                nc.vector.scalar_tensor_tensor(out=ug[:, cs], in0=ud[:, cs],
                                               scalar=2.0, in1=ub[:, cs],
                                               op0=OP.mult, op1=OP.add)
                nc.vector.tensor_tensor(out=un2[:, cs], in0=ug[:, cs],
                                        in1=uf[:, cs], op=OP.add)
                # stage 3: gamma
                un = G_("un")
                nc.scalar.activation(un[:, cs], un2[:, cs], AF.Sqrt)
                ung = G_("ung")
                nc.vector.tensor_scalar_max(ung[:, cs], un[:, cs], EPS)
                urec = G_("urec")
                nc.vector.reciprocal(urec[:, cs], ung[:, cs])
                h2n = G_("h2n")
                nc.vector.tensor_scalar_min(h2n[:, cs], un[:, cs], MN)
                at2u = artanh2("s3", h2n, cs)
                h2ng = G_("h2ng")
                nc.vector.tensor_scalar_max(h2ng[:, cs], h2n[:, cs], EPS)
                hrec = G_("hrec")
                nc.vector.reciprocal(hrec[:, cs], h2ng[:, cs])
                lam2 = G_("lam2")
                nc.vector.tensor_tensor(out=lam2[:, cs], in0=at2u[:, cs],
                                        in1=hrec[:, cs], op=OP.mult)
                pst = G_("pst")
                nc.vector.tensor_scalar(out=pst[:, cs], in0=urec[:, cs],
                                        scalar1=MN, scalar2=1.0,
                                        op0=OP.mult, op1=OP.min)
                gm0 = G_("gm0")
                nc.vector.scalar_tensor_tensor(out=gm0[:, cs],
                                               in0=lam2[:, cs], scalar=0.5,
                                               in1=pst[:, cs],
                                               op0=OP.mult, op1=OP.mult)
                gam = G_("gam")
                nc.vector.tensor_tensor(out=gam[:, cs], in0=gm0[:, cs],
                                        in1=disg[:, cs], op=OP.mult)
                # ts tiles out
                for t in trng:
                    tst = workp.tile([P, D], EX, name="tst", tag="tst")
                    nc.vector.tensor_scalar_mul(tst[:], h_grid[:, tsl(t)],
                                                gam[:, t:t + 1])
                    nc.sync.dma_start(
                        out=ts_loc[l][t * P:(t + 1) * P, :], in_=tst[:])
                if q % 2 == 1:
                    h = q // 2
                    def emit_ag(l=l, h=h):
                        nc.gpsimd.collective_compute(
                            "AllGather", OP.bypass,
                            replica_groups=[list(range(NCORES))],
                            ins=[ts_loc[l][h * NPH:(h + 1) * NPH, :].opt()],
                            outs=[ts_full[l][h][:, :].opt()])
                    if defer_ag or (l == 0 and h == 1):
                        pending_ag.append(emit_ag)
                    else:
                        emit_ag()

            # gather-plan bookkeeping: gathers per (block, half)
            gath_bh = {}
            goff16 = 0
            for (b, h, j, ni, cg) in gplan:
                gath_bh.setdefault((b, h), []).append((j, ni, cg, goff16))
                goff16 += ni // 16

            def phase_B(l):
                an2 = G_("an2")
                qctr = [0]
                mtiles = {}
                stiles = {}
                pa_g = {}

                def emit_gather(b, h):
                    for (j, ni, cg, off16) in gath_bh[(b, h)]:
                        nj = ni // P
                        m = msgp.tile([P, CPG * D], EX, name="m", tag="m")
                        nc.gpsimd.dma_gather(
                            m[:, :nj * D].rearrange("p (c e) -> p c e", c=nj),
                            ts_full[l][h],
                            gidx_sb[:, off16:off16 + ni // 16],
                            ni, cg, D, queue_num=qctr[0] % 4)
                        qctr[0] += 1
                        mtiles[(b, h, j)] = m

                def emit_S(b, h):
                    S = sblkp.tile([P, CPGH * P], EX, name="S", tag="S")
                    nch = CBH[b][h]
                    co = eoff[b] + (CBH[b][0] if h else 0)
                    nc.vector.tensor_tensor(
                        out=S[:, :nch * P].rearrange("p (c j) -> p c j",
                                                     c=nch),
                        in0=edst_sb[:, co:co + nch].to_broadcast([P, nch, P]),
                        in1=iota_sb[:].rearrange("p (o j) -> p o j", o=1)
                            .to_broadcast([P, nch, P]),
                        op=OP.is_equal)
                    stiles[(b, h)] = S

                srounds = [(b, h) for sg in range(T // G) for h in range(2)
                           for b in range(sg * G, sg * G + G)]
                sidx = [0]

                def build_S_upto(k):
                    while sidx[0] <= k and sidx[0] < len(srounds):
                        emit_S(*srounds[sidx[0]])
                        sidx[0] += 1

                i = 0
                for sg in range(T // G):
                    b0 = sg * G
                    for h in range(2):
                        for b in range(b0, b0 + G):
                            emit_gather(b, h)
                        flush_ag()
                        for b in range(b0, b0 + G):
                            build_S_upto(i + 2)
                            S = stiles.pop((b, h))
                            if h == 0:
                                pa = psAp.tile([P, D], f32, name="pa",
                                               tag="pa")
                                pa_g[b] = pa
                            else:
                                pa = pa_g[b]
                            nch = CBH[b][h]
                            for c in range(nch):
                                j, s = divmod(c, CPG)
                                m = mtiles[(b, h, j)]
                                nc.tensor.matmul(
                                    pa[:],
                                    lhsT=S[:, c * P:(c + 1) * P],
                                    rhs=m[:, s * D:(s + 1) * D],
                                    start=(h == 0 and c == 0),
                                    stop=(h == 1 and c == nch - 1))
                            for (j, _, _, _) in gath_bh[(b, h)]:
                                del mtiles[(b, h, j)]
                            i += 1
                    q = sg
                    cs = slice(q * G, (q + 1) * G)
                    for b in range(b0, b0 + G):
                        nc.vector.tensor_copy(h_grid[:, tsl(b)],
                                              pa_g.pop(b)[:])
                        sq_accum(h_grid[:, tsl(b)], an2[:, b:b + 1])
                    # grouped expmap epilogue over 5 blocks
                    n = G_("eC_n")
                    nc.scalar.activation(n[:, cs], an2[:, cs], AF.Sqrt)
                    npr = G_("eC_npr")
                    nc.vector.tensor_tensor(out=npr[:, cs], in0=n[:, cs],
                                            in1=disg[:, cs], op=OP.mult)
                    tn = G_("eC_tn")
                    nc.scalar.activation(tn[:, cs], npr[:, cs], AF.Tanh)
                    ng = G_("eC_ng")
                    nc.vector.tensor_scalar_max(ng[:, cs], npr[:, cs], EPS)
                    rec = G_("eC_rec")
                    nc.vector.reciprocal(rec[:, cs], ng[:, cs])
                    sc0 = G_("eC_sc0")
                    nc.vector.tensor_tensor(out=sc0[:, cs], in0=tn[:, cs],
                                            in1=rec[:, cs], op=OP.mult)
                    tng = G_("eC_tng")
                    nc.vector.tensor_scalar_max(tng[:, cs], tn[:, cs], EPS)
                    trec = G_("eC_trec")
                    nc.vector.reciprocal(trec[:, cs], tng[:, cs])
                    ps = G_("eC_ps")
                    nc.vector.tensor_scalar(out=ps[:, cs], in0=trec[:, cs],
                                            scalar1=MN, scalar2=1.0,
                                            op0=OP.mult, op1=OP.min)
                    sig = G_("eC_sig")
                    nc.vector.tensor_tensor(out=sig[:, cs], in0=sc0[:, cs],
                                            in1=ps[:, cs], op=OP.mult)
                    sig2 = G_("eC_sig2")
                    nc.vector.tensor_tensor(out=sig2[:, cs], in0=sig[:, cs],
                                            in1=disg[:, cs], op=OP.mult)
                    tnm = G_("eC_tnm")
                    nc.vector.tensor_scalar_min(tnm[:, cs], tn[:, cs], MN)
                    nc.vector.tensor_tensor(out=hn2[:, cs], in0=tnm[:, cs],
                                            in1=tnm[:, cs], op=OP.mult)
                    for t in range(q * G, (q + 1) * G):
                        nc.vector.tensor_scalar_mul(
                            h_grid[:, tsl(t)], h_grid[:, tsl(t)],
                            sig2[:, t:t + 1])
                    if l == 0:
                        for t in range(q * G, (q + 1) * G):
                            emit_pass1(1, t)
                        phase_A_stage(1, q, defer_ag=True)
                    else:
                        for t in range(q * G, (q + 1) * G):
                            nc.sync.dma_start(
                                out=out_ext[t * P:(t + 1) * P, :],
                                in_=h_grid[:, tsl(t)])

            # ================= init: h = expmap0(x) =================
            n2i = G_("n2i")
            for t in range(T):
                nc.sync.dma_start(out=h_grid[:, tsl(t)],
                                  in_=x_in[t * P:(t + 1) * P, :])
                sq_accum(h_grid[:, tsl(t)], n2i[:, t:t + 1])
            csA = slice(0, T)
            nI = G_("i_n")
            nc.scalar.activation(nI[:, csA], n2i[:, csA], AF.Sqrt)
            ngI = G_("i_ng")
            nc.vector.tensor_scalar_max(ngI[:, csA], nI[:, csA], EPS)
            tnI = G_("i_tn")
            nc.scalar.activation(tnI[:, csA], nI[:, csA], AF.Tanh)
            recI = G_("i_rec")
            nc.vector.reciprocal(recI[:, csA], ngI[:, csA])
            sc0I = G_("i_sc0")
            nc.vector.tensor_tensor(out=sc0I[:, csA], in0=tnI[:, csA],
                                    in1=recI[:, csA], op=OP.mult)
            tngI = G_("i_tng")
            nc.vector.tensor_scalar_max(tngI[:, csA], tnI[:, csA], EPS)
            trecI = G_("i_trec")
            nc.vector.reciprocal(trecI[:, csA], tngI[:, csA])
            psI = G_("i_ps")
            nc.vector.tensor_scalar(out=psI[:, csA], in0=trecI[:, csA],
                                    scalar1=MN, scalar2=1.0,
                                    op0=OP.mult, op1=OP.min)
            sigI = G_("i_sig")
            nc.vector.tensor_tensor(out=sigI[:, csA], in0=sc0I[:, csA],
                                    in1=psI[:, csA], op=OP.mult)
            tnmI = G_("i_tnm")
            nc.vector.tensor_scalar_min(tnmI[:, csA], tnI[:, csA], MN)
            nc.vector.tensor_tensor(out=hn2[:, csA], in0=tnmI[:, csA],
                                    in1=tnmI[:, csA], op=OP.mult)
            for t in range(T):
                nc.vector.tensor_scalar_mul(h_grid[:, tsl(t)],
                                            h_grid[:, tsl(t)],
                                            sigI[:, t:t + 1])

            # ================= layer 0 phase A =================
            for t in range(T):
                emit_pass1(0, t)
            for q in range(NQ):
                phase_A_stage(0, q)
            # ================= layer 0 phase B (layer 1 phase A inside) ====
            phase_B(0)
            # ================= layer 1 phase B =================
            phase_B(1)

    nc.compile()
    return nc


def _get_program(T, CB, CBH, gplan, DC):
    key = (T, CB, CBH, gplan, DC)
    if key not in _prog_cache:
        _prog_cache[key] = _build_program(T, CB, CBH, gplan, DC)
    return _prog_cache[key]


# ----------------------------------------------------------------- entry

def run(inputs, trace=False, trace_kwargs=None):
    x = np.asarray(inputs["x"], np.float32)
    ei = np.asarray(inputs["edge_index"])
    W1 = np.asarray(inputs["W1"], np.float32)
    b1 = np.asarray(inputs["b1"], np.float32)
    W2 = np.asarray(inputs["W2"], np.float32)
    b2 = np.asarray(inputs["b2"], np.float32)
    N, D = x.shape
    assert D % P == 0
    meta, per_core = _host_prep(x, ei)
    T, CB, CBH, gplan, DC = (meta["T"], meta["CB"], meta["CBH"],
                             meta["gplan"], D // P)
    n_loc = meta["n_loc"]

    wt = np.stack([np.ascontiguousarray(W1.T), np.ascontiguousarray(W2.T)])
    wt = wt.astype(ml_dtypes.bfloat16)
    y = np.stack([np.tile(_np_expmap0(b1)[None, :], (P, 1)),
                  np.tile(_np_expmap0(b2)[None, :], (P, 1))])

    nc = _get_program(T, CB, CBH, gplan, DC)
    in_maps = []
    for r in range(NCORES):
        m = dict(per_core[r])
        m["wt"] = wt
        m["y"] = y
        in_maps.append(m)

    kwargs = {}
    if trace:
        kwargs = dict(trace=True, trace_kwargs=trace_kwargs or {})
    res = run_bass_kernel_spmd(nc, in_maps, list(range(NCORES)), **kwargs)
    out = np.concatenate(
        [np.asarray(res.results[r]["out"])[:n_loc] for r in range(NCORES)],
        axis=0)
    return out, res


def kernel(**inputs):
    out, _ = run(inputs)
    return out


# revision 36
# speedup vs baseline: 1.0891x; 1.0891x over previous
"""Trainium2 Bass kernel for the 2-layer hyperbolic (Poincare ball) GCN encoder.

Strategy (8 NeuronCores, SPMD), v3 pipelined:
  - Nodes sharded across cores (2500 rows/core, padded to 2560 = 20 tiles of 128).
  - Weights replicated (bf16); dense mobius ops on the owned shard with all
    per-row reductions as [128, T] scalar grids (sum-of-squares on DVE via
    scalar_tensor_tensor accum to avoid ACT table thrash); |u|^2 after the
    mobius-add computed analytically from grid scalars.
  - Tangent features (pre-scaled by deg^-0.5 on the source side) are
    AllGathered in bf16 as two source-half collectives per layer (each into
    its own Shared tensor, satisfying the single-writer rule) so phase-B
    rounds can start after the first half arrives; AG triggers are deferred
    past gather batches to avoid Pool head-of-line blocking.
  - Edges partitioned by (destination block, source half), src-sorted;
    <=8 chunks of 128 edges per dma_gather (>1024 idxs hangs HW); trailing
    -1 indices skip padding DMA; num_idxs_reg carries the SPMD-common valid
    count (cores padded to it with dummy idx-0 rows).
  - Segment-sum on TensorE via 0/1 selection matrices (is_equal) accumulated
    in PSUM across both half-rounds; per-5-block grouped expmap epilogue
    (batched ACT scalars, Square on DVE).
  - Layer 1's whole phase A (matvec + mobius scalar stages + ts stores + AG
    halves) is emitted inside layer 0's phase-B super-group loop for overlap.
"""
import numpy as np
import ml_dtypes

import concourse.bass as bass
import concourse.bacc as bacc
import concourse.tile as tile
import concourse.mybir as mybir
from concourse.bass_utils import run_bass_kernel_spmd
from concourse.masks import make_identity

NCORES = 8
P = 128
NQ = 4               # AllGather chunks per layer
CPG = 8              # max chunks (of 128 edges) per dma_gather
MN = 1.0 - 4e-3
EPS = 1e-15
ATEPS = 1e-7

f32 = mybir.dt.float32
bf16 = mybir.dt.bfloat16
i16 = mybir.dt.int16
AF = mybir.ActivationFunctionType
OP = mybir.AluOpType

_prog_cache = {}


# ----------------------------------------------------------------- host side

def _np_expmap0(u):
    u = np.asarray(u, np.float32)
    n = max(float(np.linalg.norm(u)), EPS)
    v = (np.tanh(n) * u / n).astype(np.float32)
    nn = max(float(np.linalg.norm(v)), EPS)
    if nn > MN:
        v = (v / nn * MN).astype(np.float32)
    return v


def _host_prep(x, edge_index):
    x = np.asarray(x, np.float32)
    ei = np.asarray(edge_index)
    N, D = x.shape
    assert N % NCORES == 0
    n_loc = N // NCORES
    T = (n_loc + P - 1) // P
    n_pad = T * P
    assert T % NQ == 0
    TQ = T // NQ              # tiles per AG quarter
    NPQ = TQ * P              # rows per AG quarter
    assert NCORES * n_pad <= 32767, "indices must fit int16"

    loops = np.arange(N, dtype=ei.dtype)
    ei = np.concatenate([ei, np.stack([loops, loops])], axis=1)
    row, col = ei[0].astype(np.int64), ei[1].astype(np.int64)
    deg = np.bincount(col, minlength=N).astype(np.float32)
    dis = (deg ** -0.5).astype(np.float32)

    # global source index within source-half tensors
    # ts_full_h = [NCORES, NPH, D]; NPH = n_pad // 2
    NPH = n_pad // 2
    r_s = row // n_loc
    i_s = row % n_loc
    h_s = i_s // NPH
    gsrc = r_s * NPH + (i_s % NPH)

    dst_core = col // n_loc
    dst_blk = (col % n_loc) // P
    dst_rel = (col % n_loc) % P

    # per (core, block, src-half) edge lists
    edges = [[[None, None] for _ in range(T)] for _ in range(NCORES)]
    for r in range(NCORES):
        sel = dst_core == r
        gb = dst_blk[sel]
        gh = h_s[sel]
        gs = gsrc[sel]
        gr = dst_rel[sel]
        order = np.lexsort((gh, gb))
        gb, gh, gs, gr = gb[order], gh[order], gs[order], gr[order]
        key = gb * 2 + gh
        bounds = np.searchsorted(key, np.arange(2 * T + 1))
        for b in range(T):
            for h in range(2):
                lo, hi = bounds[b * 2 + h], bounds[b * 2 + h + 1]
                so = np.argsort(gs[lo:hi], kind="stable")  # src-sorted: HBM
                edges[r][b][h] = (gs[lo:hi][so], gr[lo:hi][so])  # locality

    L = np.array([[[len(edges[r][b][h][0]) for h in range(2)]
                   for b in range(T)] for r in range(NCORES)], np.int64)
    # chunks per (block, half); block's chunk list = half0 chunks + half1
    CBH = [[int(np.ceil(L[:, b, h].max() / P)) for h in range(2)]
           for b in range(T)]
    CB = [CBH[b][0] + CBH[b][1] for b in range(T)]

    # gathers of <= CPG chunks per (block, half): (b, h, j, num_idxs, cg)
    gplan = []
    for b in range(T):
        for h in range(2):
            for j in range((CBH[b][h] + CPG - 1) // CPG):
                nj = min(CPG, CBH[b][h] - CPG * j)
                num_idxs = nj * P
                vr = np.clip(L[:, b, h] - CPG * P * j, 0, num_idxs)
                cg = int(vr.max())
                gplan.append((b, h, j, num_idxs, cg))

    tot16 = sum(g[3] // 16 for g in gplan)
    sumCB = sum(CB)

    idx_w = np.zeros((NCORES, 128, tot16), np.int16)
    edst = np.full((NCORES, P, sumCB), -1.0, np.float32)
    eoff = np.concatenate([[0], np.cumsum(CB)]).astype(int)

    for r in range(NCORES):
        col16 = 0
        for (b, h, j, num_idxs, cg) in gplan:
            gs, gr = edges[r][b][h]
            lo = CPG * P * j
            vr = int(np.clip(len(gs) - lo, 0, num_idxs))
            lin = np.full(num_idxs, -1, np.int64)
            lin[:vr] = gs[lo:lo + vr]
            lin[vr:cg] = 0                      # dummy valid rows
            w = lin.reshape(num_idxs // 16, 16).T.astype(np.int16)
            nc16 = num_idxs // 16
            idx_w[r][:, col16:col16 + nc16] = np.tile(w, (8, 1))
            col16 += nc16
        for b in range(T):
            nch = CB[b]
            dpad = np.full(nch * P, -1.0, np.float32)
            gs0, gr0 = edges[r][b][0]
            gs1, gr1 = edges[r][b][1]
            dpad[:len(gr0)] = gr0.astype(np.float32)
            off1 = CBH[b][0] * P
            dpad[off1:off1 + len(gr1)] = gr1.astype(np.float32)
            edst[r][:, eoff[b]:eoff[b] + nch] = dpad.reshape(nch, P).T

    dis_loc = np.zeros((NCORES, P, T), np.float32)
    for r in range(NCORES):
        d = np.zeros(n_pad, np.float32)
        d[:n_loc] = dis[r * n_loc:(r + 1) * n_loc]
        dis_loc[r] = d.reshape(T, P).T

    x_loc = np.zeros((NCORES, n_pad, D), np.float32)
    for r in range(NCORES):
        x_loc[r, :n_loc] = x[r * n_loc:(r + 1) * n_loc]

    iota = np.tile(np.arange(P, dtype=np.float32)[None, :], (P, 1))
    meta = dict(N=N, D=D, n_loc=n_loc, T=T, CB=tuple(CB),
                CBH=tuple(tuple(c) for c in CBH),
                gplan=tuple(gplan), n_pad=n_pad, NPQ=NPQ)
    per_core = [dict(x=x_loc[r], dis=dis_loc[r], gidx=idx_w[r],
                     edst=edst[r].astype(ml_dtypes.bfloat16),
                     iota=iota.astype(ml_dtypes.bfloat16))
                for r in range(NCORES)]
    return meta, per_core


# --------------------------------------------------------------- device side

def _build_program(T, CB, CBH, gplan, DC):
    D = DC * P
    NPAD = T * P
    NPH = NPAD // 2
    TQ = T // NQ
    NPQ = TQ * P
    EX = bf16
    G = 5                      # phase-B epilogue group (blocks)
    assert T % G == 0 and TQ == G
    sumCB = sum(CB)
    eoff = [0]
    for c in CB:
        eoff.append(eoff[-1] + c)
    tot16 = sum(g[3] // 16 for g in gplan)
    CPGH = max(max(ch) for ch in CBH)      # chunks per (block, half)

    nc = bacc.Bacc("TRN2", target_bir_lowering=False, debug=False,
                   num_devices=NCORES, num_swdge_queues=4,
                   dynamic_dma_scratch_size=16384)

    x_in = nc.dram_tensor("x", [NPAD, D], f32, kind="ExternalInput")
    wt_in = nc.dram_tensor("wt", [2, D, D], bf16, kind="ExternalInput")
    y_in = nc.dram_tensor("y", [2, P, D], f32, kind="ExternalInput")
    iota_in = nc.dram_tensor("iota", [P, P], bf16, kind="ExternalInput")
    dis_in = nc.dram_tensor("dis", [P, T], f32, kind="ExternalInput")
    gidx_in = nc.dram_tensor("gidx", [P, tot16], i16, kind="ExternalInput")
    edst_in = nc.dram_tensor("edst", [P, sumCB], bf16, kind="ExternalInput")
    out_ext = nc.dram_tensor("out", [NPAD, D], f32, kind="ExternalOutput")

    with tile.TileContext(nc) as tc:
        with (
            tc.tile_pool(name="const", bufs=1) as constp,
            tc.tile_pool(name="grid", bufs=1) as gridp,
            tc.tile_pool(name="big", bufs=1) as bigp,
            tc.tile_pool(name="work", bufs=3) as workp,
            tc.tile_pool(name="junk", bufs=3) as junkp,
            tc.tile_pool(name="msgs", bufs=5) as msgp,
            tc.tile_pool(name="sblk", bufs=4) as sblkp,
            tc.tile_pool(name="psT", bufs=1, space="PSUM") as psTp,
            tc.tile_pool(name="psM", bufs=2, space="PSUM") as psMp,
            tc.tile_pool(name="psA", bufs=3, space="PSUM") as psAp,
            tc.tile_pool(name="dram", bufs=1, space="DRAM") as dramp,
        ):
            # ---- constants ----
            wt_sb = constp.tile([P, 2 * DC * D], bf16, name="wt", tag="wt")
            for l in range(2):
                for k in range(DC):
                    nc.sync.dma_start(
                        out=wt_sb[:, (l * DC + k) * D:(l * DC + k + 1) * D],
                        in_=wt_in[l, k * P:(k + 1) * P, :])
            y_sb = constp.tile([P, 2 * D], f32, name="y", tag="y")
            nc.sync.dma_start(out=y_sb[:, 0:D], in_=y_in[0])
            nc.sync.dma_start(out=y_sb[:, D:2 * D], in_=y_in[1])
            iota_sb = constp.tile([P, P], bf16, name="iota", tag="iota")
            nc.sync.dma_start(out=iota_sb[:], in_=iota_in[:, :])
            ident = constp.tile([P, P], f32, name="ident", tag="ident")
            make_identity(nc, ident[:])
            disg = constp.tile([P, T], f32, name="dis", tag="dis")
            nc.sync.dma_start(out=disg[:], in_=dis_in[:, :])
            gidx_sb = constp.tile([P, tot16], i16, name="gidx", tag="gidx")
            nc.sync.dma_start(out=gidx_sb[:], in_=gidx_in[:, :])
            edst_sb = constp.tile([P, sumCB], bf16, name="edst", tag="edst")
            nc.sync.dma_start(out=edst_sb[:], in_=edst_in[:, :])

            # message tiles: memset all bufs once (trailing-skip leaves stale
            # bytes; they are S-masked but must never be NaN)
            mprev = []
            for i in range(5):
                m0 = msgp.tile([P, CPG * D], EX, name="m", tag="m")
                nc.gpsimd.memset(m0[:], 0.0)
                mprev.append(m0)

            # ---- persistent big tensors ----
            h_grid = bigp.tile([P, T * D], f32, name="h", tag="h")  # h then u
            agg_grid = bigp.tile([P, T * D], bf16, name="agg", tag="agg")
            hn2 = gridp.tile([P, T], f32, name="hn2", tag="hn2")

            def G_(tag):
                return gridp.tile([P, T], f32, name=tag, tag=tag)

            def tsl(t):
                return slice(t * D, (t + 1) * D)

            def sq_accum(src_ap, accum_ap, eng=None):
                """accum = sum(src*src) along free axis, on DVE (or eng)."""
                jj = junkp.tile([P, D], f32, name="junk", tag="junk")
                (eng or nc.vector).scalar_tensor_tensor(
                    out=jj[:], in0=src_ap, scalar=1.0, in1=src_ap,
                    op0=OP.mult, op1=OP.mult, accum_out=accum_ap)

            # per-layer DRAM tensors; ts_full split by source half so each
            # Shared tensor has exactly one AllGather writer
            ts_loc = [dramp.tile([NPAD, D], EX, name="tsl%d" % l,
                                 tag="tsl%d" % l) for l in range(2)]
            ts_full = [[dramp.tile([NCORES * NPH, D], EX, addr_space="Shared",
                                   name="tsf%d_%d" % (l, h),
                                   tag="tsf%d_%d" % (l, h))
                        for h in range(2)] for l in range(2)]

            mxn2_g = [G_("mxn2_0"), G_("mxn2_1")]
            y2col = gridp.tile([P, 2], f32, name="y2col", tag="y2col")

            def emit_pass1(l, t):
                pt = psTp.tile([P, D], f32, name="pt", tag="pt")
                for k in range(DC):
                    nc.tensor.transpose(
                        out=pt[:, k * P:(k + 1) * P],
                        in_=h_grid[:, t * D + k * P: t * D + (k + 1) * P],
                        identity=ident[:])
                hT = workp.tile([P, D], bf16, name="hT", tag="hT")
                nc.scalar.copy(hT[:], pt[:])
                pm = psMp.tile([P, D], f32, name="pm", tag="pm")
                for k in range(DC):
                    nc.tensor.matmul(
                        pm[:],
                        lhsT=hT[:, k * P:(k + 1) * P],
                        rhs=wt_sb[:, (l * DC + k) * D:(l * DC + k + 1) * D],
                        start=(k == 0), stop=(k == DC - 1))
                nc.scalar.copy(agg_grid[:, tsl(t)], pm[:])
                sq_accum(agg_grid[:, tsl(t)], mxn2_g[l][:, t:t + 1])

            def artanh2(nm, xx, cs):
                """grid of 2*artanh(clip(xx)) over column slice cs"""
                xcl = G_(nm + "_xcl")
                nc.vector.tensor_scalar_min(xcl[:, cs], xx[:, cs], 1.0 - ATEPS)
                a1 = G_(nm + "_a1")
                nc.scalar.activation(a1[:, cs], xcl[:, cs], AF.Ln,
                                     bias=1.0, scale=1.0)
                omx = G_(nm + "_omx")
                nc.vector.tensor_scalar(out=omx[:, cs], in0=xcl[:, cs],
                                        scalar1=-1.0, scalar2=1.0,
                                        op0=OP.mult, op1=OP.add)
                a2 = G_(nm + "_a2")
                nc.scalar.activation(a2[:, cs], omx[:, cs], AF.Ln)
                at2 = G_(nm + "_at2")
                nc.vector.tensor_tensor(out=at2[:, cs], in0=a1[:, cs],
                                        in1=a2[:, cs], op=OP.subtract)
                return at2

            pending_ag = []

            def flush_ag():
                while pending_ag:
                    pending_ag.pop(0)()

            def phase_A_stage(l, q, defer_ag=False):
                """mobius-add scalar stages + passes 2/3 + ts stores + AG
                chunk, for tiles [5q, 5q+5) of layer l."""
                cs = slice(q * G, (q + 1) * G)
                trng = range(q * G, (q + 1) * G)
                y_ap = y_sb[:, l * D:(l + 1) * D]
                mxn2 = mxn2_g[l]
                if q == 0:
                    sq_accum(y_ap, y2col[:, l:l + 1])
                # stage 1
                xn = G_("xn")
                nc.scalar.activation(xn[:, cs], hn2[:, cs], AF.Sqrt)
                mxn = G_("mxn")
                nc.scalar.activation(mxn[:, cs], mxn2[:, cs], AF.Sqrt)
                xng = G_("xng")
                nc.vector.tensor_scalar_max(xng[:, cs], xn[:, cs], EPS)
                xrec = G_("xrec")
                nc.vector.reciprocal(xrec[:, cs], xng[:, cs])
                at2 = artanh2("s1", xn, cs)
                rr2 = G_("rr2")
                nc.vector.tensor_tensor(out=rr2[:, cs], in0=at2[:, cs],
                                        in1=xrec[:, cs], op=OP.mult)
                mxng = G_("mxng")
                nc.vector.tensor_scalar_max(mxng[:, cs], mxn[:, cs], EPS)
                mrec = G_("mrec")
                nc.vector.reciprocal(mrec[:, cs], mxng[:, cs])
                cc = G_("cc")
                nc.vector.scalar_tensor_tensor(out=cc[:, cs], in0=mxn[:, cs],
                                               scalar=0.5, in1=rr2[:, cs],
                                               op0=OP.mult, op1=OP.mult)
                tch = G_("tch")
                nc.scalar.activation(tch[:, cs], cc[:, cs], AF.Tanh)
                tcg = G_("tcg")
                nc.vector.tensor_scalar_max(tcg[:, cs], tch[:, cs], EPS)
                tcrec = G_("tcrec")
                nc.vector.reciprocal(tcrec[:, cs], tcg[:, cs])
                psA_ = G_("psA")
                nc.vector.tensor_scalar(out=psA_[:, cs], in0=tcrec[:, cs],
                                        scalar1=MN, scalar2=1.0,
                                        op0=OP.mult, op1=OP.min)
                sp0 = G_("sp0")
                nc.vector.tensor_tensor(out=sp0[:, cs], in0=tch[:, cs],
                                        in1=mrec[:, cs], op=OP.mult)
                spg = G_("spg")
                nc.vector.tensor_tensor(out=spg[:, cs], in0=sp0[:, cs],
                                        in1=psA_[:, cs], op=OP.mult)
                tcm = G_("tcm")
                nc.vector.tensor_scalar_min(tcm[:, cs], tch[:, cs], MN)
                x2 = G_("x2")
                nc.vector.tensor_tensor(out=x2[:, cs], in0=tcm[:, cs],
                                        in1=tcm[:, cs], op=OP.mult)
                # pass 2: xy = sum((sp*mx) . y)
                xy = G_("xy")
                for t in trng:
                    jx = junkp.tile([P, D], f32, name="junk", tag="junk")
                    nc.vector.scalar_tensor_tensor(
                        out=jx[:], in0=agg_grid[:, tsl(t)],
                        scalar=spg[:, t:t + 1], in1=y_ap,
                        op0=OP.mult, op1=OP.mult, accum_out=xy[:, t:t + 1])
                # stage 2
                t0 = G_("t0")
                nc.vector.tensor_scalar(out=t0[:, cs], in0=xy[:, cs],
                                        scalar1=2.0, scalar2=1.0,
                                        op0=OP.mult, op1=OP.add)
                ag = G_("ag")
                nc.vector.tensor_scalar_add(ag[:, cs], t0[:, cs],
                                            y2col[:, l:l + 1])
                d0 = G_("d0")
                nc.vector.tensor_scalar_mul(d0[:, cs], x2[:, cs],
                                            y2col[:, l:l + 1])
                d1 = G_("d1")
                nc.vector.tensor_tensor(out=d1[:, cs], in0=d0[:, cs],
                                        in1=t0[:, cs], op=OP.add)
                dg = G_("dg")
                nc.vector.tensor_scalar_max(dg[:, cs], d1[:, cs], EPS)
                dinv = G_("dinv")
                nc.vector.reciprocal(dinv[:, cs], dg[:, cs])
                alpha = G_("alpha")
                nc.vector.tensor_tensor(out=alpha[:, cs], in0=ag[:, cs],
                                        in1=dinv[:, cs], op=OP.mult)
                bsc = G_("bsc")
                nc.vector.tensor_scalar(out=bsc[:, cs], in0=x2[:, cs],
                                        scalar1=-1.0, scalar2=1.0,
                                        op0=OP.mult, op1=OP.add)
                beta = G_("beta")
                nc.vector.tensor_tensor(out=beta[:, cs], in0=bsc[:, cs],
                                        in1=dinv[:, cs], op=OP.mult)
                alphasp = G_("alphasp")
                nc.vector.tensor_tensor(out=alphasp[:, cs], in0=alpha[:, cs],
                                        in1=spg[:, cs], op=OP.mult)
                # pass 3: u = alphasp*mx + beta*y (into h_grid)
                for t in trng:
                    t1 = workp.tile([P, D], f32, name="t1", tag="t1")
                    nc.vector.tensor_scalar_mul(t1[:], y_ap, beta[:, t:t + 1])
                    us = h_grid[:, tsl(t)]
                    nc.vector.scalar_tensor_tensor(
                        out=us, in0=agg_grid[:, tsl(t)],
                        scalar=alphasp[:, t:t + 1], in1=t1[:],
                        op0=OP.mult, op1=OP.add)
                # |u|^2 analytically: asp^2*|mx|^2 + 2*a*b*(sp*mx.y) + b^2*|y|^2
                un2 = G_("un2")
                ua = G_("ua")
                nc.vector.tensor_tensor(out=ua[:, cs], in0=alphasp[:, cs],
                                        in1=alphasp[:, cs], op=OP.mult)
                ub = G_("ub")
                nc.vector.tensor_tensor(out=ub[:, cs], in0=ua[:, cs],
                                        in1=mxn2[:, cs], op=OP.mult)
                uc = G_("uc")
                nc.vector.tensor_tensor(out=uc[:, cs], in0=alpha[:, cs],
                                        in1=beta[:, cs], op=OP.mult)
                ud = G_("ud")
                nc.vector.tensor_tensor(out=ud[:, cs], in0=uc[:, cs],
                                        in1=xy[:, cs], op=OP.mult)
                ue = G_("ue")
                nc.vector.tensor_tensor(out=ue[:, cs], in0=beta[:, cs],
                                        in1=beta[:, cs], op=OP.mult)
                uf = G_("uf")
                nc.vector.tensor_scalar_mul(uf[:, cs], ue[:, cs],
                                            y2col[:, l:l + 1])
                ug = G_("ug")
                nc.vector.scalar_tensor_tensor(out=ug[:, cs], in0=ud[:, cs],
                                               scalar=2.0, in1=ub[:, cs],
                                               op0=OP.mult, op1=OP.add)
                nc.vector.tensor_tensor(out=un2[:, cs], in0=ug[:, cs],
                                        in1=uf[:, cs], op=OP.add)
                # stage 3: gamma
                un = G_("un")
                nc.scalar.activation(un[:, cs], un2[:, cs], AF.Sqrt)
                ung = G_("ung")
                nc.vector.tensor_scalar_max(ung[:, cs], un[:, cs], EPS)
                urec = G_("urec")
                nc.vector.reciprocal(urec[:, cs], ung[:, cs])
                h2n = G_("h2n")
                nc.vector.tensor_scalar_min(h2n[:, cs], un[:, cs], MN)
                at2u = artanh2("s3", h2n, cs)
                h2ng = G_("h2ng")
                nc.vector.tensor_scalar_max(h2ng[:, cs], h2n[:, cs], EPS)
                hrec = G_("hrec")
                nc.vector.reciprocal(hrec[:, cs], h2ng[:, cs])
                lam2 = G_("lam2")
                nc.vector.tensor_tensor(out=lam2[:, cs], in0=at2u[:, cs],
                                        in1=hrec[:, cs], op=OP.mult)
                pst = G_("pst")
                nc.vector.tensor_scalar(out=pst[:, cs], in0=urec[:, cs],
                                        scalar1=MN, scalar2=1.0,
                                        op0=OP.mult, op1=OP.min)
                gm0 = G_("gm0")
                nc.vector.scalar_tensor_tensor(out=gm0[:, cs],
                                               in0=lam2[:, cs], scalar=0.5,
                                               in1=pst[:, cs],
                                               op0=OP.mult, op1=OP.mult)
                gam = G_("gam")
                nc.vector.tensor_tensor(out=gam[:, cs], in0=gm0[:, cs],
                                        in1=disg[:, cs], op=OP.mult)
                # ts tiles out
                for t in trng:
                    tst = workp.tile([P, D], EX, name="tst", tag="tst")
                    nc.vector.tensor_scalar_mul(tst[:], h_grid[:, tsl(t)],
                                                gam[:, t:t + 1])
                    nc.sync.dma_start(
                        out=ts_loc[l][t * P:(t + 1) * P, :], in_=tst[:])
                if q % 2 == 1:
                    h = q // 2
                    def emit_ag(l=l, h=h):
                        nc.gpsimd.collective_compute(
                            "AllGather", OP.bypass,
                            replica_groups=[list(range(NCORES))],
                            ins=[ts_loc[l][h * NPH:(h + 1) * NPH, :].opt()],
                            outs=[ts_full[l][h][:, :].opt()])
                    if defer_ag or (l == 0 and h == 1):
                        pending_ag.append(emit_ag)
                    else:
                        emit_ag()

            # gather-plan bookkeeping: gathers per (block, half)
            gath_bh = {}
            goff16 = 0
            for (b, h, j, ni, cg) in gplan:
                gath_bh.setdefault((b, h), []).append((j, ni, cg, goff16))
                goff16 += ni // 16

            def phase_B(l):
                an2 = G_("an2")
                qctr = [0]
                mtiles = {}
                stiles = {}
                pa_g = {}

                def emit_gather(b, h):
                    for (j, ni, cg, off16) in gath_bh[(b, h)]:
                        nj = ni // P
                        m = msgp.tile([P, CPG * D], EX, name="m", tag="m")
                        nc.gpsimd.dma_gather(
                            m[:, :nj * D].rearrange("p (c e) -> p c e", c=nj),
                            ts_full[l][h],
                            gidx_sb[:, off16:off16 + ni // 16],
                            ni, cg, D, queue_num=qctr[0] % 4)
                        qctr[0] += 1
                        mtiles[(b, h, j)] = m

                def emit_S(b, h):
                    S = sblkp.tile([P, CPGH * P], EX, name="S", tag="S")
                    nch = CBH[b][h]
                    co = eoff[b] + (CBH[b][0] if h else 0)
                    nc.vector.tensor_tensor(
                        out=S[:, :nch * P].rearrange("p (c j) -> p c j",
                                                     c=nch),
                        in0=edst_sb[:, co:co + nch].to_broadcast([P, nch, P]),
                        in1=iota_sb[:].rearrange("p (o j) -> p o j", o=1)
                            .to_broadcast([P, nch, P]),
                        op=OP.is_equal)
                    stiles[(b, h)] = S

                srounds = [(b, h) for h in range(2)
                           for b in range(T)]
                sidx = [0]

                def build_S_upto(k):
                    while sidx[0] <= k and sidx[0] < len(srounds):
                        emit_S(*srounds[sidx[0]])
                        sidx[0] += 1

                i = 0
                for h in range(2):
                  for sg in range(T // G):
                    b0 = sg * G
                    for b in range(b0, b0 + G):
                        emit_gather(b, h)
                    flush_ag()
                    for b in range(b0, b0 + G):
                        build_S_upto(i + 2)
                        S = stiles.pop((b, h))
                        pa = psAp.tile([P, D], f32, name="pa", tag="pa")
                        nch = CBH[b][h]
                        for c in range(nch):
                            j, s = divmod(c, CPG)
                            m = mtiles[(b, h, j)]
                            nc.tensor.matmul(
                                pa[:],
                                lhsT=S[:, c * P:(c + 1) * P],
                                rhs=m[:, s * D:(s + 1) * D],
                                start=(c == 0),
                                stop=(c == nch - 1))
                        for (j, _, _, _) in gath_bh[(b, h)]:
                            del mtiles[(b, h, j)]
                        if h == 0:
                            nc.vector.tensor_copy(h_grid[:, tsl(b)], pa[:])
                        else:
                            nc.vector.tensor_tensor(
                                out=h_grid[:, tsl(b)], in0=pa[:],
                                in1=h_grid[:, tsl(b)], op=OP.add)
                        i += 1
                    if h == 0:
                        continue
                    q = sg
                    cs = slice(q * G, (q + 1) * G)
                    for b in range(b0, b0 + G):
                        sq_accum(h_grid[:, tsl(b)], an2[:, b:b + 1])
                    # grouped expmap epilogue over 5 blocks
                    n = G_("eC_n")
                    nc.scalar.activation(n[:, cs], an2[:, cs], AF.Sqrt)
                    npr = G_("eC_npr")
                    nc.vector.tensor_tensor(out=npr[:, cs], in0=n[:, cs],
                                            in1=disg[:, cs], op=OP.mult)
                    tn = G_("eC_tn")
                    nc.scalar.activation(tn[:, cs], npr[:, cs], AF.Tanh)
                    ng = G_("eC_ng")
                    nc.vector.tensor_scalar_max(ng[:, cs], npr[:, cs], EPS)
                    rec = G_("eC_rec")
                    nc.vector.reciprocal(rec[:, cs], ng[:, cs])
                    sc0 = G_("eC_sc0")
                    nc.vector.tensor_tensor(out=sc0[:, cs], in0=tn[:, cs],
                                            in1=rec[:, cs], op=OP.mult)
                    tng = G_("eC_tng")
                    nc.vector.tensor_scalar_max(tng[:, cs], tn[:, cs], EPS)
                    trec = G_("eC_trec")
                    nc.vector.reciprocal(trec[:, cs], tng[:, cs])
                    ps = G_("eC_ps")
                    nc.vector.tensor_scalar(out=ps[:, cs], in0=trec[:, cs],
                                            scalar1=MN, scalar2=1.0,
                                            op0=OP.mult, op1=OP.min)
                    sig = G_("eC_sig")
                    nc.vector.tensor_tensor(out=sig[:, cs], in0=sc0[:, cs],
                                            in1=ps[:, cs], op=OP.mult)
                    sig2 = G_("eC_sig2")
                    nc.vector.tensor_tensor(out=sig2[:, cs], in0=sig[:, cs],
                                            in1=disg[:, cs], op=OP.mult)
                    tnm = G_("eC_tnm")
                    nc.vector.tensor_scalar_min(tnm[:, cs], tn[:, cs], MN)
                    nc.vector.tensor_tensor(out=hn2[:, cs], in0=tnm[:, cs],
                                            in1=tnm[:, cs], op=OP.mult)
                    for t in range(q * G, (q + 1) * G):
                        nc.vector.tensor_scalar_mul(
                            h_grid[:, tsl(t)], h_grid[:, tsl(t)],
                            sig2[:, t:t + 1])
                    if l == 0:
                        for t in range(q * G, (q + 1) * G):
                            emit_pass1(1, t)
                        phase_A_stage(1, q, defer_ag=True)
                    else:
                        for t in range(q * G, (q + 1) * G):
                            nc.sync.dma_start(
                                out=out_ext[t * P:(t + 1) * P, :],
                                in_=h_grid[:, tsl(t)])

            # ================= init: h = expmap0(x) =================
            n2i = G_("n2i")
            for t in range(T):
                nc.sync.dma_start(out=h_grid[:, tsl(t)],
                                  in_=x_in[t * P:(t + 1) * P, :])
                sq_accum(h_grid[:, tsl(t)], n2i[:, t:t + 1])
            csA = slice(0, T)
            nI = G_("i_n")
            nc.scalar.activation(nI[:, csA], n2i[:, csA], AF.Sqrt)
            ngI = G_("i_ng")
            nc.vector.tensor_scalar_max(ngI[:, csA], nI[:, csA], EPS)
            tnI = G_("i_tn")
            nc.scalar.activation(tnI[:, csA], nI[:, csA], AF.Tanh)
            recI = G_("i_rec")
            nc.vector.reciprocal(recI[:, csA], ngI[:, csA])
            sc0I = G_("i_sc0")
            nc.vector.tensor_tensor(out=sc0I[:, csA], in0=tnI[:, csA],
                                    in1=recI[:, csA], op=OP.mult)
            tngI = G_("i_tng")
            nc.vector.tensor_scalar_max(tngI[:, csA], tnI[:, csA], EPS)
            trecI = G_("i_trec")
            nc.vector.reciprocal(trecI[:, csA], tngI[:, csA])
            psI = G_("i_ps")
            nc.vector.tensor_scalar(out=psI[:, csA], in0=trecI[:, csA],
                                    scalar1=MN, scalar2=1.0,
                                    op0=OP.mult, op1=OP.min)
            sigI = G_("i_sig")
            nc.vector.tensor_tensor(out=sigI[:, csA], in0=sc0I[:, csA],
                                    in1=psI[:, csA], op=OP.mult)
            tnmI = G_("i_tnm")
            nc.vector.tensor_scalar_min(tnmI[:, csA], tnI[:, csA], MN)
            nc.vector.tensor_tensor(out=hn2[:, csA], in0=tnmI[:, csA],
                                    in1=tnmI[:, csA], op=OP.mult)
            for t in range(T):
                nc.vector.tensor_scalar_mul(h_grid[:, tsl(t)],
                                            h_grid[:, tsl(t)],
                                            sigI[:, t:t + 1])

            # ================= layer 0 phase A =================
            for t in range(T):
                emit_pass1(0, t)
            for q in range(NQ):
                phase_A_stage(0, q)
            # ================= layer 0 phase B (layer 1 phase A inside) ====
            phase_B(0)
            # ================= layer 1 phase B =================
            phase_B(1)

    nc.compile()
    return nc


def _get_program(T, CB, CBH, gplan, DC):
    key = (T, CB, CBH, gplan, DC)
    if key not in _prog_cache:
        _prog_cache[key] = _build_program(T, CB, CBH, gplan, DC)
    return _prog_cache[key]


# ----------------------------------------------------------------- entry

def run(inputs, trace=False, trace_kwargs=None):
    x = np.asarray(inputs["x"], np.float32)
    ei = np.asarray(inputs["edge_index"])
    W1 = np.asarray(inputs["W1"], np.float32)
    b1 = np.asarray(inputs["b1"], np.float32)
    W2 = np.asarray(inputs["W2"], np.float32)
    b2 = np.asarray(inputs["b2"], np.float32)
    N, D = x.shape
    assert D % P == 0
    meta, per_core = _host_prep(x, ei)
    T, CB, CBH, gplan, DC = (meta["T"], meta["CB"], meta["CBH"],
                             meta["gplan"], D // P)
    n_loc = meta["n_loc"]

    wt = np.stack([np.ascontiguousarray(W1.T), np.ascontiguousarray(W2.T)])
    wt = wt.astype(ml_dtypes.bfloat16)
    y = np.stack([np.tile(_np_expmap0(b1)[None, :], (P, 1)),
                  np.tile(_np_expmap0(b2)[None, :], (P, 1))])

    nc = _get_program(T, CB, CBH, gplan, DC)
    in_maps = []
    for r in range(NCORES):
        m = dict(per_core[r])
        m["wt"] = wt
        m["y"] = y
        in_maps.append(m)

    kwargs = {}
    if trace:
        kwargs = dict(trace=True, trace_kwargs=trace_kwargs or {})
    res = run_bass_kernel_spmd(nc, in_maps, list(range(NCORES)), **kwargs)
    out = np.concatenate(
        [np.asarray(res.results[r]["out"])[:n_loc] for r in range(NCORES)],
        axis=0)
    return out, res


def kernel(**inputs):
    out, _ = run(inputs)
    return out


# revision 37
# speedup vs baseline: 1.0941x; 1.0046x over previous
"""Trainium2 Bass kernel for the 2-layer hyperbolic (Poincare ball) GCN encoder.

Strategy (8 NeuronCores, SPMD), v3 pipelined:
  - Nodes sharded across cores (2500 rows/core, padded to 2560 = 20 tiles of 128).
  - Weights replicated (bf16); dense mobius ops on the owned shard with all
    per-row reductions as [128, T] scalar grids (sum-of-squares on DVE via
    scalar_tensor_tensor accum to avoid ACT table thrash); |u|^2 after the
    mobius-add computed analytically from grid scalars.
  - Tangent features (pre-scaled by deg^-0.5 on the source side) are
    AllGathered in bf16 as two source-half collectives per layer (each into
    its own Shared tensor, satisfying the single-writer rule) so phase-B
    rounds can start after the first half arrives; AG triggers are deferred
    past gather batches to avoid Pool head-of-line blocking.
  - Edges partitioned by (destination block, source half), src-sorted;
    <=8 chunks of 128 edges per dma_gather (>1024 idxs hangs HW); trailing
    -1 indices skip padding DMA; num_idxs_reg carries the SPMD-common valid
    count (cores padded to it with dummy idx-0 rows).
  - Segment-sum on TensorE via 0/1 selection matrices (is_equal) accumulated
    in PSUM across both half-rounds; per-5-block grouped expmap epilogue
    (batched ACT scalars, Square on DVE).
  - Layer 1's whole phase A (matvec + mobius scalar stages + ts stores + AG
    halves) is emitted inside layer 0's phase-B super-group loop for overlap.
"""
import numpy as np
import ml_dtypes

import concourse.bass as bass
import concourse.bacc as bacc
import concourse.tile as tile
import concourse.mybir as mybir
from concourse.bass_utils import run_bass_kernel_spmd
from concourse.masks import make_identity

NCORES = 8
P = 128
NQ = 4               # AllGather chunks per layer
CPG = 8              # max chunks (of 128 edges) per dma_gather
MN = 1.0 - 4e-3
EPS = 1e-15
ATEPS = 1e-7

f32 = mybir.dt.float32
bf16 = mybir.dt.bfloat16
i16 = mybir.dt.int16
AF = mybir.ActivationFunctionType
OP = mybir.AluOpType

_prog_cache = {}


# ----------------------------------------------------------------- host side

def _np_expmap0(u):
    u = np.asarray(u, np.float32)
    n = max(float(np.linalg.norm(u)), EPS)
    v = (np.tanh(n) * u / n).astype(np.float32)
    nn = max(float(np.linalg.norm(v)), EPS)
    if nn > MN:
        v = (v / nn * MN).astype(np.float32)
    return v


def _host_prep(x, edge_index):
    x = np.asarray(x, np.float32)
    ei = np.asarray(edge_index)
    N, D = x.shape
    assert N % NCORES == 0
    n_loc = N // NCORES
    T = (n_loc + P - 1) // P
    n_pad = T * P
    assert T % NQ == 0
    TQ = T // NQ              # tiles per AG quarter
    NPQ = TQ * P              # rows per AG quarter
    assert NCORES * n_pad <= 32767, "indices must fit int16"

    loops = np.arange(N, dtype=ei.dtype)
    ei = np.concatenate([ei, np.stack([loops, loops])], axis=1)
    row, col = ei[0].astype(np.int64), ei[1].astype(np.int64)
    deg = np.bincount(col, minlength=N).astype(np.float32)
    dis = (deg ** -0.5).astype(np.float32)

    # global source index within source-half tensors
    # ts_full_h = [NCORES, NPH, D]; NPH = n_pad // 2
    NPH = n_pad // 2
    r_s = row // n_loc
    i_s = row % n_loc
    h_s = i_s // NPH
    gsrc = r_s * NPH + (i_s % NPH)

    dst_core = col // n_loc
    dst_blk = (col % n_loc) // P
    dst_rel = (col % n_loc) % P

    # per (core, block, src-half) edge lists
    edges = [[[None, None] for _ in range(T)] for _ in range(NCORES)]
    for r in range(NCORES):
        sel = dst_core == r
        gb = dst_blk[sel]
        gh = h_s[sel]
        gs = gsrc[sel]
        gr = dst_rel[sel]
        order = np.lexsort((gh, gb))
        gb, gh, gs, gr = gb[order], gh[order], gs[order], gr[order]
        key = gb * 2 + gh
        bounds = np.searchsorted(key, np.arange(2 * T + 1))
        for b in range(T):
            for h in range(2):
                lo, hi = bounds[b * 2 + h], bounds[b * 2 + h + 1]
                so = np.argsort(gs[lo:hi], kind="stable")  # src-sorted: HBM
                edges[r][b][h] = (gs[lo:hi][so], gr[lo:hi][so])  # locality

    L = np.array([[[len(edges[r][b][h][0]) for h in range(2)]
                   for b in range(T)] for r in range(NCORES)], np.int64)
    # chunks per (block, half); block's chunk list = half0 chunks + half1
    CBH = [[int(np.ceil(L[:, b, h].max() / P)) for h in range(2)]
           for b in range(T)]
    CB = [CBH[b][0] + CBH[b][1] for b in range(T)]

    # gathers of <= CPG chunks per (block, half): (b, h, j, num_idxs, cg)
    gplan = []
    for b in range(T):
        for h in range(2):
            for j in range((CBH[b][h] + CPG - 1) // CPG):
                nj = min(CPG, CBH[b][h] - CPG * j)
                num_idxs = nj * P
                vr = np.clip(L[:, b, h] - CPG * P * j, 0, num_idxs)
                cg = int(vr.max())
                gplan.append((b, h, j, num_idxs, cg))

    tot16 = sum(g[3] // 16 for g in gplan)
    sumCB = sum(CB)

    idx_w = np.zeros((NCORES, 128, tot16), np.int16)
    edst = np.full((NCORES, P, sumCB), -1.0, np.float32)
    eoff = np.concatenate([[0], np.cumsum(CB)]).astype(int)

    for r in range(NCORES):
        col16 = 0
        for (b, h, j, num_idxs, cg) in gplan:
            gs, gr = edges[r][b][h]
            lo = CPG * P * j
            vr = int(np.clip(len(gs) - lo, 0, num_idxs))
            lin = np.full(num_idxs, -1, np.int64)
            lin[:vr] = gs[lo:lo + vr]
            lin[vr:cg] = 0                      # dummy valid rows
            w = lin.reshape(num_idxs // 16, 16).T.astype(np.int16)
            nc16 = num_idxs // 16
            idx_w[r][:, col16:col16 + nc16] = np.tile(w, (8, 1))
            col16 += nc16
        for b in range(T):
            nch = CB[b]
            dpad = np.full(nch * P, -1.0, np.float32)
            gs0, gr0 = edges[r][b][0]
            gs1, gr1 = edges[r][b][1]
            dpad[:len(gr0)] = gr0.astype(np.float32)
            off1 = CBH[b][0] * P
            dpad[off1:off1 + len(gr1)] = gr1.astype(np.float32)
            edst[r][:, eoff[b]:eoff[b] + nch] = dpad.reshape(nch, P).T

    dis_loc = np.zeros((NCORES, P, T), np.float32)
    for r in range(NCORES):
        d = np.zeros(n_pad, np.float32)
        d[:n_loc] = dis[r * n_loc:(r + 1) * n_loc]
        dis_loc[r] = d.reshape(T, P).T

    x_loc = np.zeros((NCORES, n_pad, D), np.float32)
    for r in range(NCORES):
        x_loc[r, :n_loc] = x[r * n_loc:(r + 1) * n_loc]

    iota = np.tile(np.arange(P, dtype=np.float32)[None, :], (P, 1))
    meta = dict(N=N, D=D, n_loc=n_loc, T=T, CB=tuple(CB),
                CBH=tuple(tuple(c) for c in CBH),
                gplan=tuple(gplan), n_pad=n_pad, NPQ=NPQ)
    per_core = [dict(x=x_loc[r], dis=dis_loc[r], gidx=idx_w[r],
                     edst=edst[r].astype(ml_dtypes.bfloat16),
                     iota=iota.astype(ml_dtypes.bfloat16))
                for r in range(NCORES)]
    return meta, per_core


# --------------------------------------------------------------- device side

def _build_program(T, CB, CBH, gplan, DC):
    D = DC * P
    NPAD = T * P
    NPH = NPAD // 2
    TQ = T // NQ
    NPQ = TQ * P
    EX = bf16
    G = 5                      # phase-B epilogue group (blocks)
    assert T % G == 0 and TQ == G
    sumCB = sum(CB)
    eoff = [0]
    for c in CB:
        eoff.append(eoff[-1] + c)
    tot16 = sum(g[3] // 16 for g in gplan)
    CPGH = max(max(ch) for ch in CBH)      # chunks per (block, half)

    nc = bacc.Bacc("TRN2", target_bir_lowering=False, debug=False,
                   num_devices=NCORES, num_swdge_queues=4,
                   dynamic_dma_scratch_size=16384)

    x_in = nc.dram_tensor("x", [NPAD, D], f32, kind="ExternalInput")
    wt_in = nc.dram_tensor("wt", [2, D, D], bf16, kind="ExternalInput")
    y_in = nc.dram_tensor("y", [2, P, D], f32, kind="ExternalInput")
    iota_in = nc.dram_tensor("iota", [P, P], bf16, kind="ExternalInput")
    dis_in = nc.dram_tensor("dis", [P, T], f32, kind="ExternalInput")
    gidx_in = nc.dram_tensor("gidx", [P, tot16], i16, kind="ExternalInput")
    edst_in = nc.dram_tensor("edst", [P, sumCB], bf16, kind="ExternalInput")
    out_ext = nc.dram_tensor("out", [NPAD, D], f32, kind="ExternalOutput")

    with tile.TileContext(nc) as tc:
        with (
            tc.tile_pool(name="const", bufs=1) as constp,
            tc.tile_pool(name="grid", bufs=1) as gridp,
            tc.tile_pool(name="big", bufs=1) as bigp,
            tc.tile_pool(name="work", bufs=3) as workp,
            tc.tile_pool(name="junk", bufs=3) as junkp,
            tc.tile_pool(name="msgs", bufs=5) as msgp,
            tc.tile_pool(name="sblk", bufs=4) as sblkp,
            tc.tile_pool(name="psT", bufs=1, space="PSUM") as psTp,
            tc.tile_pool(name="psM", bufs=2, space="PSUM") as psMp,
            tc.tile_pool(name="psA", bufs=3, space="PSUM") as psAp,
            tc.tile_pool(name="dram", bufs=1, space="DRAM") as dramp,
        ):
            # ---- constants ----
            wt_sb = constp.tile([P, 2 * DC * D], bf16, name="wt", tag="wt")
            for l in range(2):
                for k in range(DC):
                    nc.sync.dma_start(
                        out=wt_sb[:, (l * DC + k) * D:(l * DC + k + 1) * D],
                        in_=wt_in[l, k * P:(k + 1) * P, :])
            y_sb = constp.tile([P, 2 * D], f32, name="y", tag="y")
            nc.sync.dma_start(out=y_sb[:, 0:D], in_=y_in[0])
            nc.sync.dma_start(out=y_sb[:, D:2 * D], in_=y_in[1])
            iota_sb = constp.tile([P, P], bf16, name="iota", tag="iota")
            nc.sync.dma_start(out=iota_sb[:], in_=iota_in[:, :])
            ident = constp.tile([P, P], f32, name="ident", tag="ident")
            make_identity(nc, ident[:])
            disg = constp.tile([P, T], f32, name="dis", tag="dis")
            nc.sync.dma_start(out=disg[:], in_=dis_in[:, :])
            gidx_sb = constp.tile([P, tot16], i16, name="gidx", tag="gidx")
            nc.sync.dma_start(out=gidx_sb[:], in_=gidx_in[:, :])
            edst_sb = constp.tile([P, sumCB], bf16, name="edst", tag="edst")
            nc.sync.dma_start(out=edst_sb[:], in_=edst_in[:, :])

            # message tiles: memset all bufs once (trailing-skip leaves stale
            # bytes; they are S-masked but must never be NaN)
            mprev = []
            for i in range(5):
                m0 = msgp.tile([P, CPG * D], EX, name="m", tag="m")
                nc.gpsimd.memset(m0[:], 0.0)
                mprev.append(m0)

            # ---- persistent big tensors ----
            h_grid = bigp.tile([P, T * D], f32, name="h", tag="h")  # h then u
            agg_grid = bigp.tile([P, T * D], bf16, name="agg", tag="agg")
            hn2 = gridp.tile([P, T], f32, name="hn2", tag="hn2")

            def G_(tag):
                return gridp.tile([P, T], f32, name=tag, tag=tag)

            def tsl(t):
                return slice(t * D, (t + 1) * D)

            def sq_accum(src_ap, accum_ap, eng=None):
                """accum = sum(src*src) along free axis, on DVE (or eng)."""
                jj = junkp.tile([P, D], f32, name="junk", tag="junk")
                (eng or nc.vector).scalar_tensor_tensor(
                    out=jj[:], in0=src_ap, scalar=1.0, in1=src_ap,
                    op0=OP.mult, op1=OP.mult, accum_out=accum_ap)

            # per-layer DRAM tensors; ts_full split by source half so each
            # Shared tensor has exactly one AllGather writer
            ts_loc = [dramp.tile([NPAD, D], EX, name="tsl%d" % l,
                                 tag="tsl%d" % l) for l in range(2)]
            ts_full = [[dramp.tile([NCORES * NPH, D], EX, addr_space="Shared",
                                   name="tsf%d_%d" % (l, h),
                                   tag="tsf%d_%d" % (l, h))
                        for h in range(2)] for l in range(2)]

            mxn2_g = [G_("mxn2_0"), G_("mxn2_1")]
            y2col = gridp.tile([P, 2], f32, name="y2col", tag="y2col")

            def emit_pass1(l, t):
                pt = psTp.tile([P, D], f32, name="pt", tag="pt")
                for k in range(DC):
                    nc.tensor.transpose(
                        out=pt[:, k * P:(k + 1) * P],
                        in_=h_grid[:, t * D + k * P: t * D + (k + 1) * P],
                        identity=ident[:])
                hT = workp.tile([P, D], bf16, name="hT", tag="hT")
                nc.scalar.copy(hT[:], pt[:])
                pm = psMp.tile([P, D], f32, name="pm", tag="pm")
                for k in range(DC):
                    nc.tensor.matmul(
                        pm[:],
                        lhsT=hT[:, k * P:(k + 1) * P],
                        rhs=wt_sb[:, (l * DC + k) * D:(l * DC + k + 1) * D],
                        start=(k == 0), stop=(k == DC - 1))
                nc.scalar.copy(agg_grid[:, tsl(t)], pm[:])
                sq_accum(agg_grid[:, tsl(t)], mxn2_g[l][:, t:t + 1])

            def artanh2(nm, xx, cs):
                """grid of 2*artanh(clip(xx)) over column slice cs"""
                xcl = G_(nm + "_xcl")
                nc.vector.tensor_scalar_min(xcl[:, cs], xx[:, cs], 1.0 - ATEPS)
                a1 = G_(nm + "_a1")
                nc.scalar.activation(a1[:, cs], xcl[:, cs], AF.Ln,
                                     bias=1.0, scale=1.0)
                omx = G_(nm + "_omx")
                nc.vector.tensor_scalar(out=omx[:, cs], in0=xcl[:, cs],
                                        scalar1=-1.0, scalar2=1.0,
                                        op0=OP.mult, op1=OP.add)
                a2 = G_(nm + "_a2")
                nc.scalar.activation(a2[:, cs], omx[:, cs], AF.Ln)
                at2 = G_(nm + "_at2")
                nc.vector.tensor_tensor(out=at2[:, cs], in0=a1[:, cs],
                                        in1=a2[:, cs], op=OP.subtract)
                return at2

            pending_ag = []

            def flush_ag():
                while pending_ag:
                    pending_ag.pop(0)()

            def phase_A_stage(l, q, defer_ag=False):
                """mobius-add scalar stages + passes 2/3 + ts stores + AG
                chunk, for tiles [5q, 5q+5) of layer l."""
                cs = slice(q * G, (q + 1) * G)
                trng = range(q * G, (q + 1) * G)
                y_ap = y_sb[:, l * D:(l + 1) * D]
                mxn2 = mxn2_g[l]
                if q == 0:
                    sq_accum(y_ap, y2col[:, l:l + 1])
                # stage 1
                xn = G_("xn")
                nc.scalar.activation(xn[:, cs], hn2[:, cs], AF.Sqrt)
                mxn = G_("mxn")
                nc.scalar.activation(mxn[:, cs], mxn2[:, cs], AF.Sqrt)
                xng = G_("xng")
                nc.vector.tensor_scalar_max(xng[:, cs], xn[:, cs], EPS)
                xrec = G_("xrec")
                nc.vector.reciprocal(xrec[:, cs], xng[:, cs])
                at2 = artanh2("s1", xn, cs)
                rr2 = G_("rr2")
                nc.vector.tensor_tensor(out=rr2[:, cs], in0=at2[:, cs],
                                        in1=xrec[:, cs], op=OP.mult)
                mxng = G_("mxng")
                nc.vector.tensor_scalar_max(mxng[:, cs], mxn[:, cs], EPS)
                mrec = G_("mrec")
                nc.vector.reciprocal(mrec[:, cs], mxng[:, cs])
                cc = G_("cc")
                nc.vector.scalar_tensor_tensor(out=cc[:, cs], in0=mxn[:, cs],
                                               scalar=0.5, in1=rr2[:, cs],
                                               op0=OP.mult, op1=OP.mult)
                tch = G_("tch")
                nc.scalar.activation(tch[:, cs], cc[:, cs], AF.Tanh)
                tcg = G_("tcg")
                nc.vector.tensor_scalar_max(tcg[:, cs], tch[:, cs], EPS)
                tcrec = G_("tcrec")
                nc.vector.reciprocal(tcrec[:, cs], tcg[:, cs])
                psA_ = G_("psA")
                nc.vector.tensor_scalar(out=psA_[:, cs], in0=tcrec[:, cs],
                                        scalar1=MN, scalar2=1.0,
                                        op0=OP.mult, op1=OP.min)
                sp0 = G_("sp0")
                nc.vector.tensor_tensor(out=sp0[:, cs], in0=tch[:, cs],
                                        in1=mrec[:, cs], op=OP.mult)
                spg = G_("spg")
                nc.vector.tensor_tensor(out=spg[:, cs], in0=sp0[:, cs],
                                        in1=psA_[:, cs], op=OP.mult)
                tcm = G_("tcm")
                nc.vector.tensor_scalar_min(tcm[:, cs], tch[:, cs], MN)
                x2 = G_("x2")
                nc.vector.tensor_tensor(out=x2[:, cs], in0=tcm[:, cs],
                                        in1=tcm[:, cs], op=OP.mult)
                # pass 2: xy = sum((sp*mx) . y)
                xy = G_("xy")
                for t in trng:
                    jx = junkp.tile([P, D], f32, name="junk", tag="junk")
                    nc.vector.scalar_tensor_tensor(
                        out=jx[:], in0=agg_grid[:, tsl(t)],
                        scalar=spg[:, t:t + 1], in1=y_ap,
                        op0=OP.mult, op1=OP.mult, accum_out=xy[:, t:t + 1])
                # stage 2
                t0 = G_("t0")
                nc.vector.tensor_scalar(out=t0[:, cs], in0=xy[:, cs],
                                        scalar1=2.0, scalar2=1.0,
                                        op0=OP.mult, op1=OP.add)
                ag = G_("ag")
                nc.vector.tensor_scalar_add(ag[:, cs], t0[:, cs],
                                            y2col[:, l:l + 1])
                d0 = G_("d0")
                nc.vector.tensor_scalar_mul(d0[:, cs], x2[:, cs],
                                            y2col[:, l:l + 1])
                d1 = G_("d1")
                nc.vector.tensor_tensor(out=d1[:, cs], in0=d0[:, cs],
                                        in1=t0[:, cs], op=OP.add)
                dg = G_("dg")
                nc.vector.tensor_scalar_max(dg[:, cs], d1[:, cs], EPS)
                dinv = G_("dinv")
                nc.vector.reciprocal(dinv[:, cs], dg[:, cs])
                alpha = G_("alpha")
                nc.vector.tensor_tensor(out=alpha[:, cs], in0=ag[:, cs],
                                        in1=dinv[:, cs], op=OP.mult)
                bsc = G_("bsc")
                nc.vector.tensor_scalar(out=bsc[:, cs], in0=x2[:, cs],
                                        scalar1=-1.0, scalar2=1.0,
                                        op0=OP.mult, op1=OP.add)
                beta = G_("beta")
                nc.vector.tensor_tensor(out=beta[:, cs], in0=bsc[:, cs],
                                        in1=dinv[:, cs], op=OP.mult)
                alphasp = G_("alphasp")
                nc.vector.tensor_tensor(out=alphasp[:, cs], in0=alpha[:, cs],
                                        in1=spg[:, cs], op=OP.mult)
                # pass 3: u = alphasp*mx + beta*y (into h_grid)
                for t in trng:
                    t1 = workp.tile([P, D], f32, name="t1", tag="t1")
                    nc.vector.tensor_scalar_mul(t1[:], y_ap, beta[:, t:t + 1])
                    us = h_grid[:, tsl(t)]
                    nc.vector.scalar_tensor_tensor(
                        out=us, in0=agg_grid[:, tsl(t)],
                        scalar=alphasp[:, t:t + 1], in1=t1[:],
                        op0=OP.mult, op1=OP.add)
                # |u|^2 analytically: asp^2*|mx|^2 + 2*a*b*(sp*mx.y) + b^2*|y|^2
                un2 = G_("un2")
                ua = G_("ua")
                nc.vector.tensor_tensor(out=ua[:, cs], in0=alphasp[:, cs],
                                        in1=alphasp[:, cs], op=OP.mult)
                ub = G_("ub")
                nc.vector.tensor_tensor(out=ub[:, cs], in0=ua[:, cs],
                                        in1=mxn2[:, cs], op=OP.mult)
                uc = G_("uc")
                nc.vector.tensor_tensor(out=uc[:, cs], in0=alpha[:, cs],
                                        in1=beta[:, cs], op=OP.mult)
                ud = G_("ud")
                nc.vector.tensor_tensor(out=ud[:, cs], in0=uc[:, cs],
                                        in1=xy[:, cs], op=OP.mult)
                ue = G_("ue")
                nc.vector.tensor_tensor(out=ue[:, cs], in0=beta[:, cs],
                                        in1=beta[:, cs], op=OP.mult)
                uf = G_("uf")
                nc.vector.tensor_scalar_mul(uf[:, cs], ue[:, cs],
                                            y2col[:, l:l + 1])
                ug = G_("ug")
                nc.vector.scalar_tensor_tensor(out=ug[:, cs], in0=ud[:, cs],
                                               scalar=2.0, in1=ub[:, cs],
                                               op0=OP.mult, op1=OP.add)
                nc.vector.tensor_tensor(out=un2[:, cs], in0=ug[:, cs],
                                        in1=uf[:, cs], op=OP.add)
                # stage 3: gamma
                un = G_("un")
                nc.scalar.activation(un[:, cs], un2[:, cs], AF.Sqrt)
                ung = G_("ung")
                nc.vector.tensor_scalar_max(ung[:, cs], un[:, cs], EPS)
                urec = G_("urec")
                nc.vector.reciprocal(urec[:, cs], ung[:, cs])
                h2n = G_("h2n")
                nc.vector.tensor_scalar_min(h2n[:, cs], un[:, cs], MN)
                at2u = artanh2("s3", h2n, cs)
                h2ng = G_("h2ng")
                nc.vector.tensor_scalar_max(h2ng[:, cs], h2n[:, cs], EPS)
                hrec = G_("hrec")
                nc.vector.reciprocal(hrec[:, cs], h2ng[:, cs])
                lam2 = G_("lam2")
                nc.vector.tensor_tensor(out=lam2[:, cs], in0=at2u[:, cs],
                                        in1=hrec[:, cs], op=OP.mult)
                pst = G_("pst")
                nc.vector.tensor_scalar(out=pst[:, cs], in0=urec[:, cs],
                                        scalar1=MN, scalar2=1.0,
                                        op0=OP.mult, op1=OP.min)
                gm0 = G_("gm0")
                nc.vector.scalar_tensor_tensor(out=gm0[:, cs],
                                               in0=lam2[:, cs], scalar=0.5,
                                               in1=pst[:, cs],
                                               op0=OP.mult, op1=OP.mult)
                gam = G_("gam")
                nc.vector.tensor_tensor(out=gam[:, cs], in0=gm0[:, cs],
                                        in1=disg[:, cs], op=OP.mult)
                # ts tiles out
                for t in trng:
                    tst = workp.tile([P, D], EX, name="tst", tag="tst")
                    nc.vector.tensor_scalar_mul(tst[:], h_grid[:, tsl(t)],
                                                gam[:, t:t + 1])
                    nc.sync.dma_start(
                        out=ts_loc[l][t * P:(t + 1) * P, :], in_=tst[:])
                if q % 2 == 1:
                    h = q // 2
                    def emit_ag(l=l, h=h):
                        nc.gpsimd.collective_compute(
                            "AllGather", OP.bypass,
                            replica_groups=[list(range(NCORES))],
                            ins=[ts_loc[l][h * NPH:(h + 1) * NPH, :].opt()],
                            outs=[ts_full[l][h][:, :].opt()])
                    if defer_ag or (l == 0 and h == 1):
                        pending_ag.append(emit_ag)
                    else:
                        emit_ag()

            # gather-plan bookkeeping: gathers per (block, half)
            gath_bh = {}
            goff16 = 0
            for (b, h, j, ni, cg) in gplan:
                gath_bh.setdefault((b, h), []).append((j, ni, cg, goff16))
                goff16 += ni // 16

            def phase_B(l):
                # fire any pending AG trigger now: Pool idles at the next
                # sweep's gather waits anyway, and an early trigger reduces
                # worst-of-8-cores skew at the collective engine
                flush_ag()
                an2 = G_("an2")
                qctr = [0]
                mtiles = {}
                stiles = {}
                pa_g = {}

                def emit_gather(b, h):
                    for (j, ni, cg, off16) in gath_bh[(b, h)]:
                        nj = ni // P
                        m = msgp.tile([P, CPG * D], EX, name="m", tag="m")
                        nc.gpsimd.dma_gather(
                            m[:, :nj * D].rearrange("p (c e) -> p c e", c=nj),
                            ts_full[l][h],
                            gidx_sb[:, off16:off16 + ni // 16],
                            ni, cg, D, queue_num=qctr[0] % 4)
                        qctr[0] += 1
                        mtiles[(b, h, j)] = m

                def emit_S(b, h):
                    S = sblkp.tile([P, CPGH * P], EX, name="S", tag="S")
                    nch = CBH[b][h]
                    co = eoff[b] + (CBH[b][0] if h else 0)
                    nc.vector.tensor_tensor(
                        out=S[:, :nch * P].rearrange("p (c j) -> p c j",
                                                     c=nch),
                        in0=edst_sb[:, co:co + nch].to_broadcast([P, nch, P]),
                        in1=iota_sb[:].rearrange("p (o j) -> p o j", o=1)
                            .to_broadcast([P, nch, P]),
                        op=OP.is_equal)
                    stiles[(b, h)] = S

                srounds = [(b, h) for h in range(2)
                           for b in range(T)]
                sidx = [0]

                def build_S_upto(k):
                    while sidx[0] <= k and sidx[0] < len(srounds):
                        emit_S(*srounds[sidx[0]])
                        sidx[0] += 1

                i = 0
                for h in range(2):
                  for sg in range(T // G):
                    b0 = sg * G
                    for b in range(b0, b0 + G):
                        emit_gather(b, h)
                    flush_ag()
                    for b in range(b0, b0 + G):
                        build_S_upto(i + 2)
                        S = stiles.pop((b, h))
                        pa = psAp.tile([P, D], f32, name="pa", tag="pa")
                        nch = CBH[b][h]
                        for c in range(nch):
                            j, s = divmod(c, CPG)
                            m = mtiles[(b, h, j)]
                            nc.tensor.matmul(
                                pa[:],
                                lhsT=S[:, c * P:(c + 1) * P],
                                rhs=m[:, s * D:(s + 1) * D],
                                start=(c == 0),
                                stop=(c == nch - 1))
                        for (j, _, _, _) in gath_bh[(b, h)]:
                            del mtiles[(b, h, j)]
                        if h == 0:
                            nc.vector.tensor_copy(h_grid[:, tsl(b)], pa[:])
                        else:
                            nc.vector.tensor_tensor(
                                out=h_grid[:, tsl(b)], in0=pa[:],
                                in1=h_grid[:, tsl(b)], op=OP.add)
                        i += 1
                    if h == 0:
                        continue
                    q = sg
                    cs = slice(q * G, (q + 1) * G)
                    for b in range(b0, b0 + G):
                        sq_accum(h_grid[:, tsl(b)], an2[:, b:b + 1])
                    # grouped expmap epilogue over 5 blocks
                    n = G_("eC_n")
                    nc.scalar.activation(n[:, cs], an2[:, cs], AF.Sqrt)
                    npr = G_("eC_npr")
                    nc.vector.tensor_tensor(out=npr[:, cs], in0=n[:, cs],
                                            in1=disg[:, cs], op=OP.mult)
                    tn = G_("eC_tn")
                    nc.scalar.activation(tn[:, cs], npr[:, cs], AF.Tanh)
                    ng = G_("eC_ng")
                    nc.vector.tensor_scalar_max(ng[:, cs], npr[:, cs], EPS)
                    rec = G_("eC_rec")
                    nc.vector.reciprocal(rec[:, cs], ng[:, cs])
                    sc0 = G_("eC_sc0")
                    nc.vector.tensor_tensor(out=sc0[:, cs], in0=tn[:, cs],
                                            in1=rec[:, cs], op=OP.mult)
                    tng = G_("eC_tng")
                    nc.vector.tensor_scalar_max(tng[:, cs], tn[:, cs], EPS)
                    trec = G_("eC_trec")
                    nc.vector.reciprocal(trec[:, cs], tng[:, cs])
                    ps = G_("eC_ps")
                    nc.vector.tensor_scalar(out=ps[:, cs], in0=trec[:, cs],
                                            scalar1=MN, scalar2=1.0,
                                            op0=OP.mult, op1=OP.min)
                    sig = G_("eC_sig")
                    nc.vector.tensor_tensor(out=sig[:, cs], in0=sc0[:, cs],
                                            in1=ps[:, cs], op=OP.mult)
                    sig2 = G_("eC_sig2")
                    nc.vector.tensor_tensor(out=sig2[:, cs], in0=sig[:, cs],
                                            in1=disg[:, cs], op=OP.mult)
                    tnm = G_("eC_tnm")
                    nc.vector.tensor_scalar_min(tnm[:, cs], tn[:, cs], MN)
                    nc.vector.tensor_tensor(out=hn2[:, cs], in0=tnm[:, cs],
                                            in1=tnm[:, cs], op=OP.mult)
                    for t in range(q * G, (q + 1) * G):
                        nc.vector.tensor_scalar_mul(
                            h_grid[:, tsl(t)], h_grid[:, tsl(t)],
                            sig2[:, t:t + 1])
                    if l == 0:
                        for t in range(q * G, (q + 1) * G):
                            emit_pass1(1, t)
                        phase_A_stage(1, q, defer_ag=True)
                    else:
                        for t in range(q * G, (q + 1) * G):
                            nc.sync.dma_start(
                                out=out_ext[t * P:(t + 1) * P, :],
                                in_=h_grid[:, tsl(t)])

            # ================= init: h = expmap0(x) =================
            n2i = G_("n2i")
            for t in range(T):
                nc.sync.dma_start(out=h_grid[:, tsl(t)],
                                  in_=x_in[t * P:(t + 1) * P, :])
                sq_accum(h_grid[:, tsl(t)], n2i[:, t:t + 1])
            csA = slice(0, T)
            nI = G_("i_n")
            nc.scalar.activation(nI[:, csA], n2i[:, csA], AF.Sqrt)
            ngI = G_("i_ng")
            nc.vector.tensor_scalar_max(ngI[:, csA], nI[:, csA], EPS)
            tnI = G_("i_tn")
            nc.scalar.activation(tnI[:, csA], nI[:, csA], AF.Tanh)
            recI = G_("i_rec")
            nc.vector.reciprocal(recI[:, csA], ngI[:, csA])
            sc0I = G_("i_sc0")
            nc.vector.tensor_tensor(out=sc0I[:, csA], in0=tnI[:, csA],
                                    in1=recI[:, csA], op=OP.mult)
            tngI = G_("i_tng")
            nc.vector.tensor_scalar_max(tngI[:, csA], tnI[:, csA], EPS)
            trecI = G_("i_trec")
            nc.vector.reciprocal(trecI[:, csA], tngI[:, csA])
            psI = G_("i_ps")
            nc.vector.tensor_scalar(out=psI[:, csA], in0=trecI[:, csA],
                                    scalar1=MN, scalar2=1.0,
                                    op0=OP.mult, op1=OP.min)
            sigI = G_("i_sig")
            nc.vector.tensor_tensor(out=sigI[:, csA], in0=sc0I[:, csA],
                                    in1=psI[:, csA], op=OP.mult)
            tnmI = G_("i_tnm")
            nc.vector.tensor_scalar_min(tnmI[:, csA], tnI[:, csA], MN)
            nc.vector.tensor_tensor(out=hn2[:, csA], in0=tnmI[:, csA],
                                    in1=tnmI[:, csA], op=OP.mult)
            for t in range(T):
                nc.vector.tensor_scalar_mul(h_grid[:, tsl(t)],
                                            h_grid[:, tsl(t)],
                                            sigI[:, t:t + 1])

            # ================= layer 0 phase A =================
            for t in range(T):
                emit_pass1(0, t)
            for q in range(NQ):
                phase_A_stage(0, q)
            # ================= layer 0 phase B (layer 1 phase A inside) ====
            phase_B(0)
            # ================= layer 1 phase B =================
            phase_B(1)

    nc.compile()
    return nc


def _get_program(T, CB, CBH, gplan, DC):
    key = (T, CB, CBH, gplan, DC)
    if key not in _prog_cache:
        _prog_cache[key] = _build_program(T, CB, CBH, gplan, DC)
    return _prog_cache[key]


# ----------------------------------------------------------------- entry

def run(inputs, trace=False, trace_kwargs=None):
    x = np.asarray(inputs["x"], np.float32)
    ei = np.asarray(inputs["edge_index"])
    W1 = np.asarray(inputs["W1"], np.float32)
    b1 = np.asarray(inputs["b1"], np.float32)
    W2 = np.asarray(inputs["W2"], np.float32)
    b2 = np.asarray(inputs["b2"], np.float32)
    N, D = x.shape
    assert D % P == 0
    meta, per_core = _host_prep(x, ei)
    T, CB, CBH, gplan, DC = (meta["T"], meta["CB"], meta["CBH"],
                             meta["gplan"], D // P)
    n_loc = meta["n_loc"]

    wt = np.stack([np.ascontiguousarray(W1.T), np.ascontiguousarray(W2.T)])
    wt = wt.astype(ml_dtypes.bfloat16)
    y = np.stack([np.tile(_np_expmap0(b1)[None, :], (P, 1)),
                  np.tile(_np_expmap0(b2)[None, :], (P, 1))])

    nc = _get_program(T, CB, CBH, gplan, DC)
    in_maps = []
    for r in range(NCORES):
        m = dict(per_core[r])
        m["wt"] = wt
        m["y"] = y
        in_maps.append(m)

    kwargs = {}
    if trace:
        kwargs = dict(trace=True, trace_kwargs=trace_kwargs or {})
    res = run_bass_kernel_spmd(nc, in_maps, list(range(NCORES)), **kwargs)
    out = np.concatenate(
        [np.asarray(res.results[r]["out"])[:n_loc] for r in range(NCORES)],
        axis=0)
    return out, res


def kernel(**inputs):
    out, _ = run(inputs)
    return out
